# revision 14
# baseline (speedup 1.0000x reference)
"""Trainium2 Bass kernel for nn_Decoder_17076789969159 (gnn_message_passing).

Sharding: data-parallel over batch (2 groups of 4 cores); within a group the
permuted point axis of each space-filling-curve order is split in 4 contiguous
chunks.

v3 dataflow (per group): conv outputs of all 3 orders are scatter-added
(SWDGE dma_scatter_add) into a per-point partial-sum tensor z [N, C]; a
ReduceScatter+AllGather pair over the group completes z = sum over orders of
y_o at each original point. The next layer does ONE transpose-mode dma_gather
per tile from z (channel-major landing, no PE transposes on the gather side)
and applies bn+relu post-gather on the Scalar engine. The final stage reads
z2 at contiguous point rows (HBM-friendly) with one gather per tile.

This minimizes random-row HBM traffic (the real bottleneck): one scatter +
one gather per point per layer instead of 3 slab gathers + 3 AllGathers.

Self-contained: hardcodes all shapes from the problem spec.
"""

import os
import numpy as np
import ml_dtypes

BF16 = ml_dtypes.bfloat16

# Problem shapes (hardcoded per contract)
B, N, LL, O, KK, PAD = 2, 32768, 8192, 3, 9, 4
C = 256          # conv output channels
C1R = 304        # conv1 input channels (256 xi + 48 low)
CH1 = 384        # padded h row (3 * 128)
CLS = 13
NCORES, GRP = 8, 4
QN = N // GRP            # 8192 permuted positions per core per order
NPT = 512                # points per conv tile
NT = QN // NPT           # 16 conv tiles per (order) per core
NTS0 = N // NPT          # 64 stage0 tiles (full batch, replicated in group)
NTF = QN // NPT          # 16 final tiles (core's original-index quarter)
NGI = 640                # gathered window, padded to 128 multiple
EPS = 1e-5

_CACHE = {}


# ----------------------------------------------------------------------------
# host-side preparation
# ----------------------------------------------------------------------------

def _wrap16(vals):
    """index vector of length n (mult of 16) -> [128, n//16] int16 wrapped.

    The 16-partition wrap is replicated down all 128 partitions: each of the
    8 GpSimd Q7 cores reads its own 16-partition stripe on hardware.
    """
    v = np.asarray(vals, np.int64)
    a = v.reshape(-1, 16).T.astype(np.int16)
    return np.tile(a, (8, 1))


def _bn_affine(g, b, m, v):
    s = g / np.sqrt(v + EPS)
    return s.astype(np.float32), (b - m * s).astype(np.float32)


def _prep_shared(inp):
    sh = {}
    w1 = np.asarray(inp["w1_w"], np.float32)   # [256, 304, 9]
    w1p = np.zeros((128, KK * 3 * C), np.float32)
    for j in range(KK):
        for kc in range(3):
            ci0 = kc * 128
            ncid = min(128, C1R - ci0)
            if ncid > 0:
                blk = w1[:, ci0:ci0 + ncid, j].T  # [ncid, 256]
                w1p[:ncid, (j * 3 + kc) * C:(j * 3 + kc) * C + C] = blk
    sh["w1p"] = w1p.astype(BF16)

    w2 = np.asarray(inp["w2_w"], np.float32)   # [256, 256, 9]
    w2p = np.zeros((128, KK * 2 * C), np.float32)
    for j in range(KK):
        for kc in range(2):
            blk = w2[:, kc * 128:(kc + 1) * 128, j].T
            w2p[:, (j * 2 + kc) * C:(j * 2 + kc) * C + C] = blk
    sh["w2p"] = w2p.astype(BF16)

    sh["c1"] = np.asarray(inp["conv1_w"], np.float32).T.astype(BF16)  # [128,48]

    ow = np.asarray(inp["out_w"], np.float32)  # [13, 256]
    owp = np.zeros((128, 2 * CLS), np.float32)
    for g in range(2):
        owp[:, g * CLS:(g + 1) * CLS] = ow[:, g * 128:(g + 1) * 128].T
    sh["outw"] = owp.astype(BF16)

    sh["eye"] = np.eye(128, dtype=BF16)

    bnv = np.zeros((128, 11), np.float32)
    s1, b1 = _bn_affine(inp["bn1_g"], inp["bn1_b"], inp["bn1_m"], inp["bn1_v"])
    bnv[:48, 0], bnv[:48, 1] = s1, b1
    sc, bc = _bn_affine(inp["bnc1_g"], inp["bnc1_b"], inp["bnc1_m"], inp["bnc1_v"])
    bc = bc + np.asarray(inp["w1_b"], np.float32) * sc
    for g in range(2):
        bnv[:, 2 + g] = sc[g * 128:(g + 1) * 128] / 3.0
        bnv[:, 4 + g] = bc[g * 128:(g + 1) * 128]
    sc2, bc2 = _bn_affine(inp["bnc2_g"], inp["bnc2_b"], inp["bnc2_m"], inp["bnc2_v"])
    bc2 = bc2 + np.asarray(inp["w2_b"], np.float32) * sc2
    for g in range(2):
        bnv[:, 6 + g] = sc2[g * 128:(g + 1) * 128] / 3.0
        bnv[:, 8 + g] = bc2[g * 128:(g + 1) * 128]
    bnv[:CLS, 10] = np.asarray(inp["out_b"], np.float32)
    sh["bnvec"] = bnv

    # interp tables: per chunk 1024 wrapped idxs (512 of i0, 512 of i1)
    pos = np.arange(N, dtype=np.float64) * ((LL - 1) / (N - 1))
    i0 = np.floor(pos).astype(np.int64)
    i1 = np.minimum(i0 + 1, LL - 1)
    t = (pos - i0).astype(np.float32)
    icols = []
    for ch in range(NTS0):
        s = slice(ch * NPT, (ch + 1) * NPT)
        icols.append(_wrap16(np.concatenate([i0[s], i1[s]])))
    sh["iidx"] = np.concatenate(icols, axis=1)           # [128, NTS0*64]
    tt = np.zeros((128, NTS0 * 4), np.float32)
    for ch in range(NTS0):
        for s in range(4):
            tt[:, ch * 4 + s] = t[ch * NPT + s * 128: ch * NPT + (s + 1) * 128]
    sh["tt"] = tt
    return sh


def _prep_core(inp, c):
    b, q = c // GRP, c % GRP
    pc = {}
    x = np.asarray(inp["x"], np.float32)
    pc["xt"] = np.ascontiguousarray(x[b].T).astype(BF16)          # [8192, 256]
    pc["llf"] = np.asarray(inp["low_level_feat"], np.float32)[b].astype(BF16)

    rot = np.asarray(inp["rotations"], np.int64)[:, b, :]          # [O, N]

    # geometry weights in permuted space, OOB taps zeroed
    coords = np.asarray(inp["coords"], np.float32)[b]              # [3, N]
    dist = np.asarray(inp["distances"], np.float32)[b]             # [O, N]
    wall = np.zeros((O, KK, N), np.float32)
    ar = np.arange(N)
    for o in range(O):
        co = coords[:, rot[o]]                                     # [3, N]
        d = dist[o]
        dp = np.pad(d, (PAD, PAD))
        cp = np.pad(co, ((0, 0), (PAD, PAD)))
        for j in range(KK):
            dd = (dp[j:j + N] - d) ** 2
            dc = ((cp[:, j:j + N] - co) ** 2).sum(0)
            w = np.exp(-(dd + dc))
            pin = ar + j - PAD
            w[(pin < 0) | (pin >= N)] = 0.0
            wall[o, j] = w
    # per-core chunk-blocked, pre-broadcast to 128 partitions
    wgt = np.zeros((O * NT, KK * NPT), np.float32)
    for o in range(O):
        for tch in range(NT):
            base = q * QN + tch * NPT
            wgt[o * NT + tch] = wall[o, :, base:base + NPT].reshape(KK * NPT)
    pc["wgt"] = np.ascontiguousarray(
        np.broadcast_to(wgt.astype(BF16)[:, None, :], (O * NT, 128, KK * NPT)))

    # gather window point-ids per (order, tile): used for L1 (from h_t) and
    # L2 (from zf1) gathers; plus scatter ids (the 512 owned slots' points)
    mar = np.arange(NGI)
    g1cols, sccols = [], []
    for o in range(O):
        for tch in range(NT):
            base = q * QN + tch * NPT
            pp = base - PAD + mar
            valid = (pp >= 0) & (pp < N)
            ppc = np.clip(pp, 0, N - 1)
            g1cols.append(_wrap16(np.where(valid, rot[o][ppc], 0)))
            sccols.append(_wrap16(rot[o][base:base + NPT]))
    pc["g1i"] = np.concatenate(g1cols, axis=1)    # [128, O*NT*40]
    pc["sci"] = np.concatenate(sccols, axis=1)    # [128, O*NT*32]

    fcols = []
    for tch in range(NTF):
        base = q * QN + tch * NPT
        fcols.append(_wrap16(base + np.arange(NPT)))
    pc["fci"] = np.concatenate(fcols, axis=1)     # [128, NTF*32]
    return pc


# ----------------------------------------------------------------------------
# device program
# ----------------------------------------------------------------------------

def _build_nc():
    import concourse.bacc as bacc
    import concourse.bass as bass
    import concourse.tile as tile
    import concourse.mybir as mybir
    from concourse.library_config import mlp

    dt = mybir.dt
    AF = mybir.ActivationFunctionType
    nocc = os.environ.get("KNOCC", "0") == "1"
    noag = nocc or os.environ.get("KNOAG", "0") == "1"
    nc = bacc.Bacc("TRN2", target_bir_lowering=False, debug=False,
                   num_devices=1 if nocc else NCORES)

    def EIN(name, shape, dty):
        return nc.dram_tensor(name, list(shape), dty, kind="ExternalInput")

    xt = EIN("xt", [LL, C], dt.bfloat16)
    llf = EIN("llf", [128, N], dt.bfloat16)
    w1p = EIN("w1p", [128, KK * 3 * C], dt.bfloat16)
    w2p = EIN("w2p", [128, KK * 2 * C], dt.bfloat16)
    c1 = EIN("c1", [128, 48], dt.bfloat16)
    outw = EIN("outw", [128, 2 * CLS], dt.bfloat16)
    eye = EIN("eye", [128, 128], dt.bfloat16)
    bnvec = EIN("bnvec", [128, 11], dt.float32)
    wgt = EIN("wgt", [O * NT, 128, KK * NPT], dt.bfloat16)
    g1i = EIN("g1i", [128, O * NT * (NGI // 16)], dt.int16)
    sci = EIN("sci", [128, O * NT * (NPT // 16)], dt.int16)
    fci = EIN("fci", [128, NTF * (NPT // 16)], dt.int16)
    iidx = EIN("iidx", [128, NTS0 * 64], dt.int16)
    tt = EIN("tt", [128, NTS0 * 4], dt.float32)

    out = nc.dram_tensor("out", [CLS, QN], dt.float32, kind="ExternalOutput")

    RG = [[0, 1, 2, 3], [4, 5, 6, 7]]
    NW = NGI // 16   # 40 idx cols per gather window
    NWS = NPT // 16  # 32 idx cols per scatter / final gather

    with tile.TileContext(nc) as tc:
        with (
            tc.tile_pool(name="dram", bufs=1, space="DRAM") as dpool,
            tc.tile_pool(name="res", bufs=1) as res,
            tc.tile_pool(name="wk", bufs=4) as wk,
            tc.tile_pool(name="wc", bufs=2) as wc,
            tc.tile_pool(name="ps", bufs=4, space="PSUM") as psp,
            tc.tile_pool(name="pt", bufs=4, space="PSUM") as ptp,
        ):
            nc.gpsimd.load_library(mlp)

            h_t = dpool.tile([N, CH1], dt.bfloat16, tag="h")
            z1 = dpool.tile([N, C], dt.bfloat16, tag="z1")
            z2 = dpool.tile([N, C], dt.bfloat16, tag="z2")
            r1 = dpool.tile([QN, C], dt.bfloat16, tag="r1")
            r2 = dpool.tile([QN, C], dt.bfloat16, tag="r2")
            zf1 = dpool.tile([N, C], dt.bfloat16, tag="zf1")
            zf2 = dpool.tile([N, C], dt.bfloat16, tag="zf2")

            # resident SBUF constants
            def LOAD(src, shape, dty, tag):
                tl = res.tile(shape, dty, tag=tag, name=tag)
                nc.sync.dma_start(tl[:], src[:])
                return tl
            w1s = LOAD(w1p, [128, KK * 3 * C], dt.bfloat16, "w1s")
            w2s = LOAD(w2p, [128, KK * 2 * C], dt.bfloat16, "w2s")
            c1s = LOAD(c1, [128, 48], dt.bfloat16, "c1s")
            ows = LOAD(outw, [128, 2 * CLS], dt.bfloat16, "ows")
            eys = LOAD(eye, [128, 128], dt.bfloat16, "eys")
            bns = LOAD(bnvec, [128, 11], dt.float32, "bns")
            g1s = LOAD(g1i, [128, O * NT * NW], dt.int16, "g1s")
            scs = LOAD(sci, [128, O * NT * NWS], dt.int16, "scs")
            fcs = LOAD(fci, [128, NTF * NWS], dt.int16, "fcs")
            iis = LOAD(iidx, [128, NTS0 * 64], dt.int16, "iis")
            tts = LOAD(tt, [128, NTS0 * 4], dt.float32, "tts")

            def rows_pm(dram_tile, base, nrows, rowlen):
                """point-major SBUF tile [128, nrows//128, rowlen] <-> dram rows."""
                return bass.AP(dram_tile.tensor, base * rowlen,
                               [[rowlen, 128], [128 * rowlen, nrows // 128],
                                [1, rowlen]])

            # ---------------- zero-init z1/z2 (overlaps stage 0) -----------
            ZR = 16  # 2048 rows per DMA
            zrs = res.tile([128, ZR, C], dt.bfloat16, tag="zrs", name="zrs")
            nc.vector.memset(zrs[:], 0)
            for zt_ in (z1, z2):
                for k in range(N // (128 * ZR)):
                    nc.sync.dma_start(rows_pm(zt_, k * 128 * ZR, 128 * ZR, C),
                                      zrs[:])

            # ---------------- stage 0: build h ----------------
            for ch in range(NTS0):
                xg = wk.tile([128, 8, C], dt.bfloat16, tag="xg", bufs=3)
                nc.gpsimd.dma_gather(
                    xg[:, :, :], xt[:, :], iis[:, ch * 64:(ch + 1) * 64],
                    8 * 128, 8 * 128, C, transpose=False)
                hrow = wk.tile([128, 4, CH1], dt.bfloat16, tag="hrow")
                xd = wk.tile([128, 4, C], dt.bfloat16, tag="xd")
                nc.vector.tensor_sub(xd[:], xg[:, 4:8, :], xg[:, 0:4, :])
                for s in range(4):
                    nc.vector.tensor_scalar_mul(
                        xd[:, s, :], xd[:, s, :], tts[:, ch * 4 + s:ch * 4 + s + 1])
                nc.vector.tensor_add(hrow[:, :, 0:C], xg[:, 0:4, :], xd[:])

                lsb = wk.tile([128, NPT], dt.bfloat16, tag="lsb")
                nc.sync.dma_start(lsb[:], llf[:, ch * NPT:(ch + 1) * NPT])
                p48 = psp.tile([48, NPT], dt.float32, tag="pc")
                nc.tensor.matmul(p48[:], c1s[:], lsb[:], start=True, stop=True)
                low = wk.tile([48, NPT], dt.bfloat16, tag="low")
                nc.scalar.activation(low[:], p48[:], AF.Relu,
                                     bias=bns[:48, 1:2], scale=bns[:48, 0:1])
                for s in range(4):
                    ptt = ptp.tile([128, 48], dt.bfloat16, tag="pt")
                    nc.tensor.transpose(ptt[:], low[:48, s * 128:(s + 1) * 128],
                                        eys[:48, :48])
                    nc.scalar.activation(hrow[:, s, C:C + 48], ptt[:], AF.Copy)
                nc.vector.memset(hrow[:, :, C + 48:CH1], 0)
                nc.sync.dma_start(rows_pm(h_t, ch * NPT, NPT, CH1), hrow[:])

            # ---------------- conv layer helper ----------------
            def conv_layer(zdst, wsb_pack, nkc, gather_one, after=None):
                for o in range(O):
                    for tch in range(NT):
                        blk = o * NT + tch
                        hx = gather_one(o, tch)
                        wsb = wc.tile([128, KK * NPT], dt.bfloat16, tag="wsb")
                        nc.sync.dma_start(wsb[:], wgt[blk, :, :])
                        pg = [psp.tile([128, NPT], dt.float32, tag="pc",
                                       name=f"pg{g}") for g in range(2)]
                        for j in range(KK):
                            xw = wk.tile([128, nkc, NPT], dt.bfloat16, tag="xw")
                            for kc in range(nkc):
                                nc.vector.tensor_mul(
                                    xw[:, kc, :], hx[:, kc, j:j + NPT],
                                    wsb[:, j * NPT:(j + 1) * NPT])
                            for g in range(2):
                                for kc in range(nkc):
                                    wsl = wsb_pack[:, ((j * nkc + kc) * C + g * 128):
                                                   ((j * nkc + kc) * C + g * 128 + 128)]
                                    nc.tensor.matmul(
                                        pg[g][:], wsl, xw[:, kc, :],
                                        start=(j == 0 and kc == 0),
                                        stop=(j == KK - 1 and kc == nkc - 1))
                        ysb = wk.tile([128, 2, NPT], dt.bfloat16, tag="ysb")
                        for g in range(2):
                            nc.scalar.activation(ysb[:, g, :], pg[g][:], AF.Copy)
                        yT = wk.tile([128, 4, C], dt.bfloat16, tag="yT")
                        for g in range(2):
                            for s in range(4):
                                ptt = ptp.tile([128, 128], dt.bfloat16, tag="pt")
                                nc.tensor.transpose(
                                    ptt[:], ysb[:, g, s * 128:(s + 1) * 128], eys[:])
                                nc.scalar.activation(
                                    yT[:, s, g * 128:(g + 1) * 128], ptt[:], AF.Copy)
                        nc.gpsimd.dma_scatter_add(
                            zdst[:, :], yT[:, :, :],
                            scs[:, blk * NWS:(blk + 1) * NWS], NPT, NPT, C)
                if after is not None:
                    after()

            # L1: transpose-gather padded-384 rows of h -> channel-major
            def gather_l1(o, tch):
                blk = o * NT + tch
                hx = wk.tile([128, 3, NGI], dt.bfloat16, tag="g1hx", bufs=6)
                nc.gpsimd.dma_gather(
                    hx[:, :, :], h_t[:, :], g1s[:, blk * NW:(blk + 1) * NW],
                    NGI, NGI, CH1, transpose=True)
                return hx

            def rsag(z, r, zf):
                if noag:
                    return
                nc.gpsimd.collective_compute(
                    "AllReduce", mybir.AluOpType.add, replica_groups=RG,
                    ins=[z.opt()], outs=[zf.opt()])

            conv_layer(z1, w1s, 3, gather_l1, after=lambda: rsag(z1, r1, zf1))

            # L2: one transpose-gather from zf1 (same point-id table as L1),
            # then bn1+relu on the scalar engine
            def gather_l2(o, tch):
                blk = o * NT + tch
                g2t = wk.tile([128, 2, NGI], dt.bfloat16, tag="g2t", bufs=6)
                nc.gpsimd.dma_gather(
                    g2t[:, :, :], zf1[:, :], g1s[:, blk * NW:(blk + 1) * NW],
                    NGI, NGI, C, transpose=True)
                hx = wk.tile([128, 2, NGI], dt.bfloat16, tag="g2hx")
                for g in range(2):
                    nc.scalar.activation(hx[:, g, :], g2t[:, g, :], AF.Relu,
                                         bias=bns[:, 4 + g:5 + g],
                                         scale=bns[:, 2 + g:3 + g])
                return hx

            conv_layer(z2, w2s, 2, gather_l2, after=lambda: rsag(z2, r2, zf2))

            # ---------------- final: bn2+relu+proj ----------------
            for tch in range(NTF):
                g3t = wk.tile([128, 2, NPT], dt.bfloat16, tag="g3t", bufs=4)
                nc.gpsimd.dma_gather(
                    g3t[:, :, :], zf2[:, :], fcs[:, tch * NWS:(tch + 1) * NWS],
                    NPT, NPT, C, transpose=True)
                h2 = wk.tile([128, 2, NPT], dt.bfloat16, tag="h2")
                for g in range(2):
                    nc.scalar.activation(h2[:, g, :], g3t[:, g, :], AF.Relu,
                                         bias=bns[:, 8 + g:9 + g],
                                         scale=bns[:, 6 + g:7 + g])
                pf = psp.tile([CLS, NPT], dt.float32, tag="pc")
                for g in range(2):
                    nc.tensor.matmul(pf[:], ows[:, g * CLS:(g + 1) * CLS],
                                     h2[:, g, :], start=(g == 0), stop=(g == 1))
                osb = wk.tile([CLS, NPT], dt.float32, tag="osb")
                nc.vector.tensor_scalar_add(osb[:], pf[:], bns[:CLS, 10:11])
                nc.sync.dma_start(out[:, tch * NPT:(tch + 1) * NPT], osb[:])

    nc.compile()
    return nc


# ----------------------------------------------------------------------------
# entry point
# ----------------------------------------------------------------------------

def kernel(**inputs):
    from concourse.bass_utils import run_bass_kernel_spmd

    if "nc" not in _CACHE:
        _CACHE["nc"] = _build_nc()
    nc = _CACHE["nc"]

    sh = _prep_shared(inputs)
    in_maps = []
    for c in range(NCORES):
        m = dict(sh)
        m.update(_prep_core(inputs, c))
        in_maps.append(m)

    res = run_bass_kernel_spmd(nc, in_maps, core_ids=list(range(NCORES)))
    outs = res.results
    full = np.zeros((B, CLS, N), np.float32)
    for c in range(NCORES):
        b, q = c // GRP, c % GRP
        full[b, :, q * QN:(q + 1) * QN] = outs[c]["out"]
    return full


# revision 15
# speedup vs baseline: 1.1558x; 1.1558x over previous
"""Trainium2 Bass kernel for nn_Decoder_17076789969159 (gnn_message_passing).

Sharding: data-parallel over batch (2 groups of 4 cores); within a group the
permuted point axis of each space-filling-curve order is split in 4 contiguous
chunks.

v3 dataflow (per group): conv outputs of all 3 orders are scatter-added
(SWDGE dma_scatter_add) into a per-point partial-sum tensor z [N, C]; a
ReduceScatter+AllGather pair over the group completes z = sum over orders of
y_o at each original point. The next layer does ONE transpose-mode dma_gather
per tile from z (channel-major landing, no PE transposes on the gather side)
and applies bn+relu post-gather on the Scalar engine. The final stage reads
z2 at contiguous point rows (HBM-friendly) with one gather per tile.

This minimizes random-row HBM traffic (the real bottleneck): one scatter +
one gather per point per layer instead of 3 slab gathers + 3 AllGathers.

Self-contained: hardcodes all shapes from the problem spec.
"""

import os
import numpy as np
import ml_dtypes

BF16 = ml_dtypes.bfloat16

# Problem shapes (hardcoded per contract)
B, N, LL, O, KK, PAD = 2, 32768, 8192, 3, 9, 4
C = 256          # conv output channels
C1R = 304        # conv1 input channels (256 xi + 48 low)
CH1 = 384        # padded h row (3 * 128)
CLS = 13
NCORES, GRP = 8, 4
QN = N // GRP            # 8192 permuted positions per core per order
NPT = 512                # points per conv tile
NT = QN // NPT           # 16 conv tiles per (order) per core
NTS0 = N // NPT          # 64 stage0 tiles (full batch, replicated in group)
NTF = QN // NPT          # 16 final tiles (core's original-index quarter)
NGI = 640                # gathered window, padded to 128 multiple
EPS = 1e-5

_CACHE = {}


# ----------------------------------------------------------------------------
# host-side preparation
# ----------------------------------------------------------------------------

def _wrap16(vals):
    """index vector of length n (mult of 16) -> [128, n//16] int16 wrapped.

    The 16-partition wrap is replicated down all 128 partitions: each of the
    8 GpSimd Q7 cores reads its own 16-partition stripe on hardware.
    """
    v = np.asarray(vals, np.int64)
    a = v.reshape(-1, 16).T.astype(np.int16)
    return np.tile(a, (8, 1))


def _bn_affine(g, b, m, v):
    s = g / np.sqrt(v + EPS)
    return s.astype(np.float32), (b - m * s).astype(np.float32)


def _prep_shared(inp):
    sh = {}
    w1 = np.asarray(inp["w1_w"], np.float32)   # [256, 304, 9]
    w1p = np.zeros((128, KK * 3 * C), np.float32)
    for j in range(KK):
        for kc in range(3):
            ci0 = kc * 128
            ncid = min(128, C1R - ci0)
            if ncid > 0:
                blk = w1[:, ci0:ci0 + ncid, j].T  # [ncid, 256]
                w1p[:ncid, (j * 3 + kc) * C:(j * 3 + kc) * C + C] = blk
    sh["w1p"] = w1p.astype(BF16)

    w2 = np.asarray(inp["w2_w"], np.float32)   # [256, 256, 9]
    w2p = np.zeros((128, KK * 2 * C), np.float32)
    for j in range(KK):
        for kc in range(2):
            blk = w2[:, kc * 128:(kc + 1) * 128, j].T
            w2p[:, (j * 2 + kc) * C:(j * 2 + kc) * C + C] = blk
    sh["w2p"] = w2p.astype(BF16)

    sh["c1"] = np.asarray(inp["conv1_w"], np.float32).T.astype(BF16)  # [128,48]

    ow = np.asarray(inp["out_w"], np.float32)  # [13, 256]
    owp = np.zeros((128, 2 * CLS), np.float32)
    for g in range(2):
        owp[:, g * CLS:(g + 1) * CLS] = ow[:, g * 128:(g + 1) * 128].T
    sh["outw"] = owp.astype(BF16)

    sh["eye"] = np.eye(128, dtype=BF16)

    bnv = np.zeros((128, 11), np.float32)
    s1, b1 = _bn_affine(inp["bn1_g"], inp["bn1_b"], inp["bn1_m"], inp["bn1_v"])
    bnv[:48, 0], bnv[:48, 1] = s1, b1
    sc, bc = _bn_affine(inp["bnc1_g"], inp["bnc1_b"], inp["bnc1_m"], inp["bnc1_v"])
    bc = bc + np.asarray(inp["w1_b"], np.float32) * sc
    for g in range(2):
        bnv[:, 2 + g] = sc[g * 128:(g + 1) * 128] / 3.0
        bnv[:, 4 + g] = bc[g * 128:(g + 1) * 128]
    sc2, bc2 = _bn_affine(inp["bnc2_g"], inp["bnc2_b"], inp["bnc2_m"], inp["bnc2_v"])
    bc2 = bc2 + np.asarray(inp["w2_b"], np.float32) * sc2
    for g in range(2):
        bnv[:, 6 + g] = sc2[g * 128:(g + 1) * 128] / 3.0
        bnv[:, 8 + g] = bc2[g * 128:(g + 1) * 128]
    bnv[:CLS, 10] = np.asarray(inp["out_b"], np.float32)
    sh["bnvec"] = bnv

    # interp tables: per chunk 1024 wrapped idxs (512 of i0, 512 of i1)
    pos = np.arange(N, dtype=np.float64) * ((LL - 1) / (N - 1))
    i0 = np.floor(pos).astype(np.int64)
    i1 = np.minimum(i0 + 1, LL - 1)
    t = (pos - i0).astype(np.float32)
    icols = []
    for ch in range(NTS0):
        s = slice(ch * NPT, (ch + 1) * NPT)
        icols.append(_wrap16(np.concatenate([i0[s], i1[s]])))
    sh["iidx"] = np.concatenate(icols, axis=1)           # [128, NTS0*64]
    tt = np.zeros((128, NTS0 * 4), np.float32)
    for ch in range(NTS0):
        for s in range(4):
            tt[:, ch * 4 + s] = t[ch * NPT + s * 128: ch * NPT + (s + 1) * 128]
    sh["tt"] = tt
    return sh


def _prep_core(inp, c):
    b, q = c // GRP, c % GRP
    pc = {}
    x = np.asarray(inp["x"], np.float32)
    pc["xt"] = np.ascontiguousarray(x[b].T).astype(BF16)          # [8192, 256]
    pc["llf"] = np.asarray(inp["low_level_feat"], np.float32)[b].astype(BF16)

    rot = np.asarray(inp["rotations"], np.int64)[:, b, :]          # [O, N]

    # geometry weights in permuted space, OOB taps zeroed
    coords = np.asarray(inp["coords"], np.float32)[b]              # [3, N]
    dist = np.asarray(inp["distances"], np.float32)[b]             # [O, N]
    wall = np.zeros((O, KK, N), np.float32)
    ar = np.arange(N)
    for o in range(O):
        co = coords[:, rot[o]]                                     # [3, N]
        d = dist[o]
        dp = np.pad(d, (PAD, PAD))
        cp = np.pad(co, ((0, 0), (PAD, PAD)))
        for j in range(KK):
            dd = (dp[j:j + N] - d) ** 2
            dc = ((cp[:, j:j + N] - co) ** 2).sum(0)
            w = np.exp(-(dd + dc))
            pin = ar + j - PAD
            w[(pin < 0) | (pin >= N)] = 0.0
            wall[o, j] = w
    # per-core chunk-blocked, pre-broadcast to 128 partitions
    wgt = np.zeros((O * NT, KK * NPT), np.float32)
    for o in range(O):
        for tch in range(NT):
            base = q * QN + tch * NPT
            wgt[o * NT + tch] = wall[o, :, base:base + NPT].reshape(KK * NPT)
    pc["wgt"] = np.ascontiguousarray(
        np.broadcast_to(wgt.astype(BF16)[:, None, :], (O * NT, 128, KK * NPT)))

    # gather window point-ids per (order, tile): used for L1 (from h_t) and
    # L2 (from zf1) gathers; plus scatter ids (the 512 owned slots' points)
    mar = np.arange(NGI)
    g1cols, sccols = [], []
    for o in range(O):
        for tch in range(NT):
            base = q * QN + tch * NPT
            pp = base - PAD + mar
            valid = (pp >= 0) & (pp < N)
            ppc = np.clip(pp, 0, N - 1)
            g1cols.append(_wrap16(np.where(valid, rot[o][ppc], 0)))
            sccols.append(_wrap16(rot[o][base:base + NPT]))
    pc["g1i"] = np.concatenate(g1cols, axis=1)    # [128, O*NT*40]
    pc["sci"] = np.concatenate(sccols, axis=1)    # [128, O*NT*32]

    fcols = []
    for tch in range(NTF):
        base = q * QN + tch * NPT
        fcols.append(_wrap16(base + np.arange(NPT)))
    pc["fci"] = np.concatenate(fcols, axis=1)     # [128, NTF*32]
    return pc


# ----------------------------------------------------------------------------
# device program
# ----------------------------------------------------------------------------

def _build_nc():
    import concourse.bacc as bacc
    import concourse.bass as bass
    import concourse.tile as tile
    import concourse.mybir as mybir
    from concourse.library_config import mlp

    dt = mybir.dt
    AF = mybir.ActivationFunctionType
    nocc = os.environ.get("KNOCC", "0") == "1"
    noag = nocc or os.environ.get("KNOAG", "0") == "1"
    nc = bacc.Bacc("TRN2", target_bir_lowering=False, debug=False,
                   num_devices=1 if nocc else NCORES)

    def EIN(name, shape, dty):
        return nc.dram_tensor(name, list(shape), dty, kind="ExternalInput")

    xt = EIN("xt", [LL, C], dt.bfloat16)
    llf = EIN("llf", [128, N], dt.bfloat16)
    w1p = EIN("w1p", [128, KK * 3 * C], dt.bfloat16)
    w2p = EIN("w2p", [128, KK * 2 * C], dt.bfloat16)
    c1 = EIN("c1", [128, 48], dt.bfloat16)
    outw = EIN("outw", [128, 2 * CLS], dt.bfloat16)
    eye = EIN("eye", [128, 128], dt.bfloat16)
    bnvec = EIN("bnvec", [128, 11], dt.float32)
    wgt = EIN("wgt", [O * NT, 128, KK * NPT], dt.bfloat16)
    g1i = EIN("g1i", [128, O * NT * (NGI // 16)], dt.int16)
    sci = EIN("sci", [128, O * NT * (NPT // 16)], dt.int16)
    fci = EIN("fci", [128, NTF * (NPT // 16)], dt.int16)
    iidx = EIN("iidx", [128, NTS0 * 64], dt.int16)
    tt = EIN("tt", [128, NTS0 * 4], dt.float32)

    out = nc.dram_tensor("out", [CLS, QN], dt.float32, kind="ExternalOutput")

    RG = [[0, 1, 2, 3], [4, 5, 6, 7]]
    NW = NGI // 16   # 40 idx cols per gather window
    NWS = NPT // 16  # 32 idx cols per scatter / final gather

    with tile.TileContext(nc) as tc:
        with (
            tc.tile_pool(name="dram", bufs=1, space="DRAM") as dpool,
            tc.tile_pool(name="res", bufs=1) as res,
            tc.tile_pool(name="wk", bufs=4) as wk,
            tc.tile_pool(name="wc", bufs=3) as wc,
            tc.tile_pool(name="ps", bufs=4, space="PSUM") as psp,
            tc.tile_pool(name="pt", bufs=4, space="PSUM") as ptp,
        ):
            nc.gpsimd.load_library(mlp)

            h_t = dpool.tile([N, CH1], dt.bfloat16, tag="h")
            z1 = dpool.tile([N, C], dt.bfloat16, tag="z1")
            z2 = dpool.tile([N, C], dt.bfloat16, tag="z2")
            r1 = dpool.tile([QN, C], dt.bfloat16, tag="r1")
            r2 = dpool.tile([QN, C], dt.bfloat16, tag="r2")
            zf1 = dpool.tile([N, C], dt.bfloat16, tag="zf1")
            zf2 = dpool.tile([N, C], dt.bfloat16, tag="zf2")

            # resident SBUF constants
            def LOAD(src, shape, dty, tag):
                tl = res.tile(shape, dty, tag=tag, name=tag)
                nc.sync.dma_start(tl[:], src[:])
                return tl
            w1s = LOAD(w1p, [128, KK * 3 * C], dt.bfloat16, "w1s")
            w2s = LOAD(w2p, [128, KK * 2 * C], dt.bfloat16, "w2s")
            c1s = LOAD(c1, [128, 48], dt.bfloat16, "c1s")
            ows = LOAD(outw, [128, 2 * CLS], dt.bfloat16, "ows")
            eys = LOAD(eye, [128, 128], dt.bfloat16, "eys")
            bns = LOAD(bnvec, [128, 11], dt.float32, "bns")
            g1s = LOAD(g1i, [128, O * NT * NW], dt.int16, "g1s")
            scs = LOAD(sci, [128, O * NT * NWS], dt.int16, "scs")
            fcs = LOAD(fci, [128, NTF * NWS], dt.int16, "fcs")
            iis = LOAD(iidx, [128, NTS0 * 64], dt.int16, "iis")
            tts = LOAD(tt, [128, NTS0 * 4], dt.float32, "tts")

            def rows_pm(dram_tile, base, nrows, rowlen):
                """point-major SBUF tile [128, nrows//128, rowlen] <-> dram rows."""
                return bass.AP(dram_tile.tensor, base * rowlen,
                               [[rowlen, 128], [128 * rowlen, nrows // 128],
                                [1, rowlen]])

            # ---------------- zero-init z1/z2 (overlaps stage 0) -----------
            ZR = 16  # 2048 rows per DMA
            zrs = res.tile([128, ZR, C], dt.bfloat16, tag="zrs", name="zrs")
            nc.vector.memset(zrs[:], 0)
            for zt_ in (z1, z2):
                for k in range(N // (128 * ZR)):
                    nc.sync.dma_start(rows_pm(zt_, k * 128 * ZR, 128 * ZR, C),
                                      zrs[:])

            # ---------------- stage 0: build h ----------------
            for ch in range(NTS0):
                xg = wk.tile([128, 8, C], dt.bfloat16, tag="xg")
                nc.gpsimd.dma_gather(
                    xg[:, :, :], xt[:, :], iis[:, ch * 64:(ch + 1) * 64],
                    8 * 128, 8 * 128, C, transpose=False)
                hrow = wk.tile([128, 4, CH1], dt.bfloat16, tag="hrow")
                xd = wk.tile([128, 4, C], dt.bfloat16, tag="xd")
                nc.vector.tensor_sub(xd[:], xg[:, 4:8, :], xg[:, 0:4, :])
                for s in range(4):
                    nc.vector.tensor_scalar_mul(
                        xd[:, s, :], xd[:, s, :], tts[:, ch * 4 + s:ch * 4 + s + 1])
                nc.vector.tensor_add(hrow[:, :, 0:C], xg[:, 0:4, :], xd[:])

                lsb = wk.tile([128, NPT], dt.bfloat16, tag="lsb")
                nc.sync.dma_start(lsb[:], llf[:, ch * NPT:(ch + 1) * NPT])
                p48 = psp.tile([48, NPT], dt.float32, tag="pc")
                nc.tensor.matmul(p48[:], c1s[:], lsb[:], start=True, stop=True)
                low = wk.tile([48, NPT], dt.bfloat16, tag="low")
                nc.scalar.activation(low[:], p48[:], AF.Relu,
                                     bias=bns[:48, 1:2], scale=bns[:48, 0:1])
                for s in range(4):
                    ptt = ptp.tile([128, 48], dt.bfloat16, tag="pt")
                    nc.tensor.transpose(ptt[:], low[:48, s * 128:(s + 1) * 128],
                                        eys[:48, :48])
                    nc.scalar.activation(hrow[:, s, C:C + 48], ptt[:], AF.Copy)
                nc.vector.memset(hrow[:, :, C + 48:CH1], 0)
                nc.sync.dma_start(rows_pm(h_t, ch * NPT, NPT, CH1), hrow[:])

            # ---------------- conv layer helper ----------------
            def conv_layer(zdst, wsb_pack, nkc, gather_one, after=None):
                for o in range(O):
                    for tch in range(NT):
                        blk = o * NT + tch
                        hx = gather_one(o, tch)
                        wsb = wc.tile([128, KK * NPT], dt.bfloat16, tag="wsb")
                        nc.sync.dma_start(wsb[:], wgt[blk, :, :])
                        pg = [psp.tile([128, NPT], dt.float32, tag="pc",
                                       name=f"pg{g}") for g in range(2)]
                        for j in range(KK):
                            xw = wk.tile([128, nkc, NPT], dt.bfloat16, tag="xw")
                            for kc in range(nkc):
                                nc.vector.tensor_mul(
                                    xw[:, kc, :], hx[:, kc, j:j + NPT],
                                    wsb[:, j * NPT:(j + 1) * NPT])
                            for g in range(2):
                                for kc in range(nkc):
                                    wsl = wsb_pack[:, ((j * nkc + kc) * C + g * 128):
                                                   ((j * nkc + kc) * C + g * 128 + 128)]
                                    nc.tensor.matmul(
                                        pg[g][:], wsl, xw[:, kc, :],
                                        start=(j == 0 and kc == 0),
                                        stop=(j == KK - 1 and kc == nkc - 1))
                        ysb = wk.tile([128, 2, NPT], dt.bfloat16, tag="ysb")
                        for g in range(2):
                            nc.scalar.activation(ysb[:, g, :], pg[g][:], AF.Copy)
                        yT = wk.tile([128, 4, C], dt.bfloat16, tag="yT")
                        for g in range(2):
                            for s in range(4):
                                ptt = ptp.tile([128, 128], dt.bfloat16, tag="pt")
                                nc.tensor.transpose(
                                    ptt[:], ysb[:, g, s * 128:(s + 1) * 128], eys[:])
                                nc.scalar.activation(
                                    yT[:, s, g * 128:(g + 1) * 128], ptt[:], AF.Copy)
                        nc.gpsimd.dma_scatter_add(
                            zdst[:, :], yT[:, :, :],
                            scs[:, blk * NWS:(blk + 1) * NWS], NPT, NPT, C)
                if after is not None:
                    after()

            # L1: transpose-gather padded-384 rows of h -> channel-major
            def gather_l1(o, tch):
                blk = o * NT + tch
                hx = wk.tile([128, 3, NGI], dt.bfloat16, tag="g1hx")
                nc.gpsimd.dma_gather(
                    hx[:, :, :], h_t[:, :], g1s[:, blk * NW:(blk + 1) * NW],
                    NGI, NGI, CH1, transpose=True)
                return hx

            def rsag(z, r, zf):
                if noag:
                    return
                nc.gpsimd.collective_compute(
                    "AllReduce", mybir.AluOpType.add, replica_groups=RG,
                    ins=[z.opt()], outs=[zf.opt()])

            conv_layer(z1, w1s, 3, gather_l1, after=lambda: rsag(z1, r1, zf1))

            # L2: one transpose-gather from zf1 (same point-id table as L1),
            # then bn1+relu on the scalar engine
            def gather_l2(o, tch):
                blk = o * NT + tch
                g2t = wk.tile([128, 2, NGI], dt.bfloat16, tag="g2t")
                nc.gpsimd.dma_gather(
                    g2t[:, :, :], zf1[:, :], g1s[:, blk * NW:(blk + 1) * NW],
                    NGI, NGI, C, transpose=True)
                hx = wk.tile([128, 2, NGI], dt.bfloat16, tag="g2hx")
                for g in range(2):
                    nc.scalar.activation(hx[:, g, :], g2t[:, g, :], AF.Relu,
                                         bias=bns[:, 4 + g:5 + g],
                                         scale=bns[:, 2 + g:3 + g])
                return hx

            conv_layer(z2, w2s, 2, gather_l2, after=lambda: rsag(z2, r2, zf2))

            # ---------------- final: bn2+relu+proj ----------------
            for tch in range(NTF):
                g3t = wk.tile([128, 2, NPT], dt.bfloat16, tag="g3t")
                nc.gpsimd.dma_gather(
                    g3t[:, :, :], zf2[:, :], fcs[:, tch * NWS:(tch + 1) * NWS],
                    NPT, NPT, C, transpose=True)
                h2 = wk.tile([128, 2, NPT], dt.bfloat16, tag="h2")
                for g in range(2):
                    nc.scalar.activation(h2[:, g, :], g3t[:, g, :], AF.Relu,
                                         bias=bns[:, 8 + g:9 + g],
                                         scale=bns[:, 6 + g:7 + g])
                pf = psp.tile([CLS, NPT], dt.float32, tag="pc")
                for g in range(2):
                    nc.tensor.matmul(pf[:], ows[:, g * CLS:(g + 1) * CLS],
                                     h2[:, g, :], start=(g == 0), stop=(g == 1))
                osb = wk.tile([CLS, NPT], dt.float32, tag="osb")
                nc.vector.tensor_scalar_add(osb[:], pf[:], bns[:CLS, 10:11])
                nc.sync.dma_start(out[:, tch * NPT:(tch + 1) * NPT], osb[:])

    nc.compile()
    return nc


# ----------------------------------------------------------------------------
# entry point
# ----------------------------------------------------------------------------

def kernel(**inputs):
    from concourse.bass_utils import run_bass_kernel_spmd

    if "nc" not in _CACHE:
        _CACHE["nc"] = _build_nc()
    nc = _CACHE["nc"]

    sh = _prep_shared(inputs)
    in_maps = []
    for c in range(NCORES):
        m = dict(sh)
        m.update(_prep_core(inputs, c))
        in_maps.append(m)

    res = run_bass_kernel_spmd(nc, in_maps, core_ids=list(range(NCORES)))
    outs = res.results
    full = np.zeros((B, CLS, N), np.float32)
    for c in range(NCORES):
        b, q = c // GRP, c % GRP
        full[b, :, q * QN:(q + 1) * QN] = outs[c]["out"]
    return full


# revision 17
# speedup vs baseline: 1.2058x; 1.0432x over previous
"""Trainium2 Bass kernel for nn_Decoder_17076789969159 (gnn_message_passing).

Sharding: data-parallel over batch (2 groups of 4 cores); within a group the
permuted point axis of each space-filling-curve order is split in 4 contiguous
chunks.

v3 dataflow (per group): conv outputs of all 3 orders are scatter-added
(SWDGE dma_scatter_add) into a per-point partial-sum tensor z [N, C]; a
ReduceScatter+AllGather pair over the group completes z = sum over orders of
y_o at each original point. The next layer does ONE transpose-mode dma_gather
per tile from z (channel-major landing, no PE transposes on the gather side)
and applies bn+relu post-gather on the Scalar engine. The final stage reads
z2 at contiguous point rows (HBM-friendly) with one gather per tile.

This minimizes random-row HBM traffic (the real bottleneck): one scatter +
one gather per point per layer instead of 3 slab gathers + 3 AllGathers.

Self-contained: hardcodes all shapes from the problem spec.
"""

import os
import numpy as np
import ml_dtypes

BF16 = ml_dtypes.bfloat16

# Problem shapes (hardcoded per contract)
B, N, LL, O, KK, PAD = 2, 32768, 8192, 3, 9, 4
C = 256          # conv output channels
C1R = 304        # conv1 input channels (256 xi + 48 low)
CH1 = 384        # padded h row (3 * 128)
CLS = 13
NCORES, GRP = 8, 4
QN = N // GRP            # 8192 permuted positions per core per order
NPT = 512                # points per conv tile
NT = QN // NPT           # 16 conv tiles per (order) per core
NTS0 = N // NPT          # 64 stage0 tiles (full batch, replicated in group)
NTF = QN // NPT          # 16 final tiles (core's original-index quarter)
NGI = 640                # gathered window, padded to 128 multiple
EPS = 1e-5

_CACHE = {}


# ----------------------------------------------------------------------------
# host-side preparation
# ----------------------------------------------------------------------------

def _wrap16(vals):
    """index vector of length n (mult of 16) -> [128, n//16] int16 wrapped.

    The 16-partition wrap is replicated down all 128 partitions: each of the
    8 GpSimd Q7 cores reads its own 16-partition stripe on hardware.
    """
    v = np.asarray(vals, np.int64)
    a = v.reshape(-1, 16).T.astype(np.int16)
    return np.tile(a, (8, 1))


def _bn_affine(g, b, m, v):
    s = g / np.sqrt(v + EPS)
    return s.astype(np.float32), (b - m * s).astype(np.float32)


def _prep_shared(inp):
    sh = {}
    w1 = np.asarray(inp["w1_w"], np.float32)   # [256, 304, 9]
    w1p = np.zeros((128, KK * 3 * C), np.float32)
    for j in range(KK):
        for kc in range(3):
            ci0 = kc * 128
            ncid = min(128, C1R - ci0)
            if ncid > 0:
                blk = w1[:, ci0:ci0 + ncid, j].T  # [ncid, 256]
                w1p[:ncid, (j * 3 + kc) * C:(j * 3 + kc) * C + C] = blk
    sh["w1p"] = w1p.astype(BF16)

    w2 = np.asarray(inp["w2_w"], np.float32)   # [256, 256, 9]
    w2p = np.zeros((128, KK * 2 * C), np.float32)
    for j in range(KK):
        for kc in range(2):
            blk = w2[:, kc * 128:(kc + 1) * 128, j].T
            w2p[:, (j * 2 + kc) * C:(j * 2 + kc) * C + C] = blk
    sh["w2p"] = w2p.astype(BF16)

    sh["c1"] = np.asarray(inp["conv1_w"], np.float32).T.astype(BF16)  # [128,48]

    ow = np.asarray(inp["out_w"], np.float32)  # [13, 256]
    owp = np.zeros((128, 2 * CLS), np.float32)
    for g in range(2):
        owp[:, g * CLS:(g + 1) * CLS] = ow[:, g * 128:(g + 1) * 128].T
    sh["outw"] = owp.astype(BF16)

    sh["eye"] = np.eye(128, dtype=BF16)

    bnv = np.zeros((128, 11), np.float32)
    s1, b1 = _bn_affine(inp["bn1_g"], inp["bn1_b"], inp["bn1_m"], inp["bn1_v"])
    bnv[:48, 0], bnv[:48, 1] = s1, b1
    sc, bc = _bn_affine(inp["bnc1_g"], inp["bnc1_b"], inp["bnc1_m"], inp["bnc1_v"])
    bc = bc + np.asarray(inp["w1_b"], np.float32) * sc
    for g in range(2):
        bnv[:, 2 + g] = sc[g * 128:(g + 1) * 128] / 3.0
        bnv[:, 4 + g] = bc[g * 128:(g + 1) * 128]
    sc2, bc2 = _bn_affine(inp["bnc2_g"], inp["bnc2_b"], inp["bnc2_m"], inp["bnc2_v"])
    bc2 = bc2 + np.asarray(inp["w2_b"], np.float32) * sc2
    for g in range(2):
        bnv[:, 6 + g] = sc2[g * 128:(g + 1) * 128] / 3.0
        bnv[:, 8 + g] = bc2[g * 128:(g + 1) * 128]
    bnv[:CLS, 10] = np.asarray(inp["out_b"], np.float32)
    sh["bnvec"] = bnv

    # interp tables: per chunk 1024 wrapped idxs (512 of i0, 512 of i1)
    pos = np.arange(N, dtype=np.float64) * ((LL - 1) / (N - 1))
    i0 = np.floor(pos).astype(np.int64)
    i1 = np.minimum(i0 + 1, LL - 1)
    t = (pos - i0).astype(np.float32)
    icols = []
    for ch in range(NTS0):
        s = slice(ch * NPT, (ch + 1) * NPT)
        icols.append(_wrap16(np.concatenate([i0[s], i1[s]])))
    sh["iidx"] = np.concatenate(icols, axis=1)           # [128, NTS0*64]
    tt = np.zeros((128, NTS0 * 4), np.float32)
    for ch in range(NTS0):
        for s in range(4):
            tt[:, ch * 4 + s] = t[ch * NPT + s * 128: ch * NPT + (s + 1) * 128]
    sh["tt"] = tt
    return sh


def _prep_core(inp, c):
    b, q = c // GRP, c % GRP
    pc = {}
    x = np.asarray(inp["x"], np.float32)
    pc["xt"] = np.ascontiguousarray(x[b].T).astype(BF16)          # [8192, 256]
    pc["llf"] = np.asarray(inp["low_level_feat"], np.float32)[b].astype(BF16)

    rot = np.asarray(inp["rotations"], np.int64)[:, b, :]          # [O, N]

    # geometry weights in permuted space, OOB taps zeroed
    coords = np.asarray(inp["coords"], np.float32)[b]              # [3, N]
    dist = np.asarray(inp["distances"], np.float32)[b]             # [O, N]
    wall = np.zeros((O, KK, N), np.float32)
    ar = np.arange(N)
    for o in range(O):
        co = coords[:, rot[o]]                                     # [3, N]
        d = dist[o]
        dp = np.pad(d, (PAD, PAD))
        cp = np.pad(co, ((0, 0), (PAD, PAD)))
        for j in range(KK):
            dd = (dp[j:j + N] - d) ** 2
            dc = ((cp[:, j:j + N] - co) ** 2).sum(0)
            w = np.exp(-(dd + dc))
            pin = ar + j - PAD
            w[(pin < 0) | (pin >= N)] = 0.0
            wall[o, j] = w
    # per-core chunk-blocked, pre-broadcast to 128 partitions
    wgt = np.zeros((O * NT, KK * NPT), np.float32)
    for o in range(O):
        for tch in range(NT):
            base = q * QN + tch * NPT
            wgt[o * NT + tch] = wall[o, :, base:base + NPT].reshape(KK * NPT)
    pc["wgt"] = np.ascontiguousarray(
        np.broadcast_to(wgt.astype(BF16)[:, None, :], (O * NT, 128, KK * NPT)))

    # gather window point-ids per (order, tile): used for L1 (from h_t) and
    # L2 (from zf1) gathers; plus scatter ids (the 512 owned slots' points)
    mar = np.arange(NGI)
    g1cols, sccols = [], []
    for o in range(O):
        for tch in range(NT):
            base = q * QN + tch * NPT
            pp = base - PAD + mar
            valid = (pp >= 0) & (pp < N)
            ppc = np.clip(pp, 0, N - 1)
            g1cols.append(_wrap16(np.where(valid, rot[o][ppc], 0)))
            sccols.append(_wrap16(rot[o][base:base + NPT]))
    pc["g1i"] = np.concatenate(g1cols, axis=1)    # [128, O*NT*40]
    pc["sci"] = np.concatenate(sccols, axis=1)    # [128, O*NT*32]

    # final gathers read the core's own ReduceScatter quarter r2 [QN, C]:
    # local row = global point - q*QN = tch*NPT + i
    fcols = []
    for tch in range(NTF):
        fcols.append(_wrap16(tch * NPT + np.arange(NPT)))
    pc["fci"] = np.concatenate(fcols, axis=1)     # [128, NTF*32]
    return pc


# ----------------------------------------------------------------------------
# device program
# ----------------------------------------------------------------------------

def _build_nc():
    import concourse.bacc as bacc
    import concourse.bass as bass
    import concourse.tile as tile
    import concourse.mybir as mybir
    from concourse.library_config import mlp

    dt = mybir.dt
    AF = mybir.ActivationFunctionType
    nocc = os.environ.get("KNOCC", "0") == "1"
    noag = nocc or os.environ.get("KNOAG", "0") == "1"
    nc = bacc.Bacc("TRN2", target_bir_lowering=False, debug=False,
                   num_devices=1 if nocc else NCORES)

    def EIN(name, shape, dty):
        return nc.dram_tensor(name, list(shape), dty, kind="ExternalInput")

    xt = EIN("xt", [LL, C], dt.bfloat16)
    llf = EIN("llf", [128, N], dt.bfloat16)
    w1p = EIN("w1p", [128, KK * 3 * C], dt.bfloat16)
    w2p = EIN("w2p", [128, KK * 2 * C], dt.bfloat16)
    c1 = EIN("c1", [128, 48], dt.bfloat16)
    outw = EIN("outw", [128, 2 * CLS], dt.bfloat16)
    eye = EIN("eye", [128, 128], dt.bfloat16)
    bnvec = EIN("bnvec", [128, 11], dt.float32)
    wgt = EIN("wgt", [O * NT, 128, KK * NPT], dt.bfloat16)
    g1i = EIN("g1i", [128, O * NT * (NGI // 16)], dt.int16)
    sci = EIN("sci", [128, O * NT * (NPT // 16)], dt.int16)
    fci = EIN("fci", [128, NTF * (NPT // 16)], dt.int16)
    iidx = EIN("iidx", [128, NTS0 * 64], dt.int16)
    tt = EIN("tt", [128, NTS0 * 4], dt.float32)

    out = nc.dram_tensor("out", [CLS, QN], dt.float32, kind="ExternalOutput")

    RG = [[0, 1, 2, 3], [4, 5, 6, 7]]
    NW = NGI // 16   # 40 idx cols per gather window
    NWS = NPT // 16  # 32 idx cols per scatter / final gather

    with tile.TileContext(nc) as tc:
        with (
            tc.tile_pool(name="dram", bufs=1, space="DRAM") as dpool,
            tc.tile_pool(name="res", bufs=1) as res,
            tc.tile_pool(name="wk", bufs=4) as wk,
            tc.tile_pool(name="wc", bufs=3) as wc,
            tc.tile_pool(name="ps", bufs=4, space="PSUM") as psp,
            tc.tile_pool(name="pt", bufs=4, space="PSUM") as ptp,
        ):
            nc.gpsimd.load_library(mlp)

            h_t = dpool.tile([N, CH1], dt.bfloat16, tag="h")
            z1 = dpool.tile([N, C], dt.bfloat16, tag="z1")
            z2 = dpool.tile([N, C], dt.bfloat16, tag="z2")
            r2 = dpool.tile([QN, C], dt.bfloat16, tag="r2")
            zf1 = dpool.tile([N, C], dt.bfloat16, tag="zf1")

            # resident SBUF constants
            def LOAD(src, shape, dty, tag):
                tl = res.tile(shape, dty, tag=tag, name=tag)
                nc.sync.dma_start(tl[:], src[:])
                return tl
            w1s = LOAD(w1p, [128, KK * 3 * C], dt.bfloat16, "w1s")
            w2s = LOAD(w2p, [128, KK * 2 * C], dt.bfloat16, "w2s")
            c1s = LOAD(c1, [128, 48], dt.bfloat16, "c1s")
            ows = LOAD(outw, [128, 2 * CLS], dt.bfloat16, "ows")
            eys = LOAD(eye, [128, 128], dt.bfloat16, "eys")
            bns = LOAD(bnvec, [128, 11], dt.float32, "bns")
            g1s = LOAD(g1i, [128, O * NT * NW], dt.int16, "g1s")
            scs = LOAD(sci, [128, O * NT * NWS], dt.int16, "scs")
            fcs = LOAD(fci, [128, NTF * NWS], dt.int16, "fcs")
            iis = LOAD(iidx, [128, NTS0 * 64], dt.int16, "iis")
            tts = LOAD(tt, [128, NTS0 * 4], dt.float32, "tts")

            def rows_pm(dram_tile, base, nrows, rowlen):
                """point-major SBUF tile [128, nrows//128, rowlen] <-> dram rows."""
                return bass.AP(dram_tile.tensor, base * rowlen,
                               [[rowlen, 128], [128 * rowlen, nrows // 128],
                                [1, rowlen]])

            # ---------------- zero-init z1/z2 (overlaps stage 0) -----------
            ZR = 16  # 2048 rows per DMA
            zrs = res.tile([128, ZR, C], dt.bfloat16, tag="zrs", name="zrs")
            nc.vector.memset(zrs[:], 0)
            for zt_ in (z1, z2):
                for k in range(N // (128 * ZR)):
                    nc.sync.dma_start(rows_pm(zt_, k * 128 * ZR, 128 * ZR, C),
                                      zrs[:])

            # ---------------- stage 0: build h ----------------
            for ch in range(NTS0):
                xg = wk.tile([128, 8, C], dt.bfloat16, tag="xg")
                nc.gpsimd.dma_gather(
                    xg[:, :, :], xt[:, :], iis[:, ch * 64:(ch + 1) * 64],
                    8 * 128, 8 * 128, C, transpose=False)
                hrow = wk.tile([128, 4, CH1], dt.bfloat16, tag="hrow")
                xd = wk.tile([128, 4, C], dt.bfloat16, tag="xd")
                nc.vector.tensor_sub(xd[:], xg[:, 4:8, :], xg[:, 0:4, :])
                for s in range(4):
                    nc.vector.tensor_scalar_mul(
                        xd[:, s, :], xd[:, s, :], tts[:, ch * 4 + s:ch * 4 + s + 1])
                nc.vector.tensor_add(hrow[:, :, 0:C], xg[:, 0:4, :], xd[:])

                lsb = wk.tile([128, NPT], dt.bfloat16, tag="lsb")
                nc.sync.dma_start(lsb[:], llf[:, ch * NPT:(ch + 1) * NPT])
                p48 = psp.tile([48, NPT], dt.float32, tag="pc")
                nc.tensor.matmul(p48[:], c1s[:], lsb[:], start=True, stop=True)
                low = wk.tile([48, NPT], dt.bfloat16, tag="low")
                nc.scalar.activation(low[:], p48[:], AF.Relu,
                                     bias=bns[:48, 1:2], scale=bns[:48, 0:1])
                for s in range(4):
                    ptt = ptp.tile([128, 48], dt.bfloat16, tag="pt")
                    nc.tensor.transpose(ptt[:], low[:48, s * 128:(s + 1) * 128],
                                        eys[:48, :48])
                    nc.scalar.activation(hrow[:, s, C:C + 48], ptt[:], AF.Copy)
                nc.vector.memset(hrow[:, :, C + 48:CH1], 0)
                nc.sync.dma_start(rows_pm(h_t, ch * NPT, NPT, CH1), hrow[:])

            # ---------------- conv layer helper ----------------
            def conv_layer(zdst, wsb_pack, nkc, gather_one, after=None):
                for o in range(O):
                    for tch in range(NT):
                        blk = o * NT + tch
                        hx = gather_one(o, tch)
                        wsb = wc.tile([128, KK * NPT], dt.bfloat16, tag="wsb")
                        nc.sync.dma_start(wsb[:], wgt[blk, :, :])
                        pg = [psp.tile([128, NPT], dt.float32, tag="pc",
                                       name=f"pg{g}") for g in range(2)]
                        for j in range(KK):
                            xw = wk.tile([128, nkc, NPT], dt.bfloat16, tag="xw")
                            for kc in range(nkc):
                                nc.vector.tensor_mul(
                                    xw[:, kc, :], hx[:, kc, j:j + NPT],
                                    wsb[:, j * NPT:(j + 1) * NPT])
                            for g in range(2):
                                for kc in range(nkc):
                                    wsl = wsb_pack[:, ((j * nkc + kc) * C + g * 128):
                                                   ((j * nkc + kc) * C + g * 128 + 128)]
                                    nc.tensor.matmul(
                                        pg[g][:], wsl, xw[:, kc, :],
                                        start=(j == 0 and kc == 0),
                                        stop=(j == KK - 1 and kc == nkc - 1))
                        ysb = wk.tile([128, 2, NPT], dt.bfloat16, tag="ysb")
                        for g in range(2):
                            nc.scalar.activation(ysb[:, g, :], pg[g][:], AF.Copy)
                        yT = wk.tile([128, 4, C], dt.bfloat16, tag="yT")
                        for g in range(2):
                            for s in range(4):
                                ptt = ptp.tile([128, 128], dt.bfloat16, tag="pt")
                                nc.tensor.transpose(
                                    ptt[:], ysb[:, g, s * 128:(s + 1) * 128], eys[:])
                                nc.scalar.activation(
                                    yT[:, s, g * 128:(g + 1) * 128], ptt[:], AF.Copy)
                        nc.gpsimd.dma_scatter_add(
                            zdst[:, :], yT[:, :, :],
                            scs[:, blk * NWS:(blk + 1) * NWS], NPT, NPT, C)
                if after is not None:
                    after()

            # L1: transpose-gather padded-384 rows of h -> channel-major
            def gather_l1(o, tch):
                blk = o * NT + tch
                hx = wk.tile([128, 3, NGI], dt.bfloat16, tag="g1hx")
                nc.gpsimd.dma_gather(
                    hx[:, :, :], h_t[:, :], g1s[:, blk * NW:(blk + 1) * NW],
                    NGI, NGI, CH1, transpose=True)
                return hx

            def ar1():
                if noag:
                    return
                nc.gpsimd.collective_compute(
                    "AllReduce", mybir.AluOpType.add, replica_groups=RG,
                    ins=[z1.opt()], outs=[zf1.opt()])

            def rs2():
                if noag:
                    return
                nc.gpsimd.collective_compute(
                    "ReduceScatter", mybir.AluOpType.add, replica_groups=RG,
                    ins=[z2.opt()], outs=[r2.opt()])

            conv_layer(z1, w1s, 3, gather_l1, after=ar1)

            # L2: one transpose-gather from zf1 (same point-id table as L1),
            # then bn1+relu on the scalar engine
            def gather_l2(o, tch):
                blk = o * NT + tch
                g2t = wk.tile([128, 2, NGI], dt.bfloat16, tag="g2t")
                nc.gpsimd.dma_gather(
                    g2t[:, :, :], zf1[:, :], g1s[:, blk * NW:(blk + 1) * NW],
                    NGI, NGI, C, transpose=True)
                hx = wk.tile([128, 2, NGI], dt.bfloat16, tag="g2hx")
                for g in range(2):
                    nc.scalar.activation(hx[:, g, :], g2t[:, g, :], AF.Relu,
                                         bias=bns[:, 4 + g:5 + g],
                                         scale=bns[:, 2 + g:3 + g])
                return hx

            conv_layer(z2, w2s, 2, gather_l2, after=rs2)

            # ---------------- final: bn2+relu+proj ----------------
            for tch in range(NTF):
                g3t = wk.tile([128, 2, NPT], dt.bfloat16, tag="g3t")
                nc.gpsimd.dma_gather(
                    g3t[:, :, :], r2[:, :], fcs[:, tch * NWS:(tch + 1) * NWS],
                    NPT, NPT, C, transpose=True)
                h2 = wk.tile([128, 2, NPT], dt.bfloat16, tag="h2")
                for g in range(2):
                    nc.scalar.activation(h2[:, g, :], g3t[:, g, :], AF.Relu,
                                         bias=bns[:, 8 + g:9 + g],
                                         scale=bns[:, 6 + g:7 + g])
                pf = psp.tile([CLS, NPT], dt.float32, tag="pc")
                for g in range(2):
                    nc.tensor.matmul(pf[:], ows[:, g * CLS:(g + 1) * CLS],
                                     h2[:, g, :], start=(g == 0), stop=(g == 1))
                osb = wk.tile([CLS, NPT], dt.float32, tag="osb")
                nc.vector.tensor_scalar_add(osb[:], pf[:], bns[:CLS, 10:11])
                nc.sync.dma_start(out[:, tch * NPT:(tch + 1) * NPT], osb[:])

    nc.compile()
    return nc


# ----------------------------------------------------------------------------
# entry point
# ----------------------------------------------------------------------------

def kernel(**inputs):
    from concourse.bass_utils import run_bass_kernel_spmd

    if "nc" not in _CACHE:
        _CACHE["nc"] = _build_nc()
    nc = _CACHE["nc"]

    sh = _prep_shared(inputs)
    in_maps = []
    for c in range(NCORES):
        m = dict(sh)
        m.update(_prep_core(inputs, c))
        in_maps.append(m)

    res = run_bass_kernel_spmd(nc, in_maps, core_ids=list(range(NCORES)))
    outs = res.results
    full = np.zeros((B, CLS, N), np.float32)
    for c in range(NCORES):
        b, q = c // GRP, c % GRP
        full[b, :, q * QN:(q + 1) * QN] = outs[c]["out"]
    return full


# revision 18
# speedup vs baseline: 1.4875x; 1.2336x over previous
"""Trainium2 Bass kernel for nn_Decoder_17076789969159 (gnn_message_passing).

Sharding: data-parallel over batch (2 groups of 4 cores); within a group the
permuted point axis of each space-filling-curve order is split in 4 contiguous
chunks.

v3 dataflow (per group): conv outputs of all 3 orders are scatter-added
(SWDGE dma_scatter_add) into a per-point partial-sum tensor z [N, C]; a
ReduceScatter+AllGather pair over the group completes z = sum over orders of
y_o at each original point. The next layer does ONE transpose-mode dma_gather
per tile from z (channel-major landing, no PE transposes on the gather side)
and applies bn+relu post-gather on the Scalar engine. The final stage reads
z2 at contiguous point rows (HBM-friendly) with one gather per tile.

This minimizes random-row HBM traffic (the real bottleneck): one scatter +
one gather per point per layer instead of 3 slab gathers + 3 AllGathers.

Self-contained: hardcodes all shapes from the problem spec.
"""

import os
import numpy as np
import ml_dtypes

BF16 = ml_dtypes.bfloat16

# Problem shapes (hardcoded per contract)
B, N, LL, O, KK, PAD = 2, 32768, 8192, 3, 9, 4
C = 256          # conv output channels
C1R = 304        # conv1 input channels (256 xi + 48 low)
CH1 = 384        # padded h row (3 * 128)
CLS = 13
NCORES, GRP = 8, 4
QN = N // GRP            # 8192 permuted positions per core per order
NPT = 512                # points per conv tile
NT = QN // NPT           # 16 conv tiles per (order) per core
NTS0 = N // NPT          # 64 stage0 tiles (full batch, replicated in group)
NTF = QN // NPT          # 16 final tiles (core's original-index quarter)
NGI = 640                # gathered window, padded to 128 multiple
EPS = 1e-5

_CACHE = {}


# ----------------------------------------------------------------------------
# host-side preparation
# ----------------------------------------------------------------------------

def _wrap16(vals):
    """index vector of length n (mult of 16) -> [128, n//16] int16 wrapped.

    The 16-partition wrap is replicated down all 128 partitions: each of the
    8 GpSimd Q7 cores reads its own 16-partition stripe on hardware.
    """
    v = np.asarray(vals, np.int64)
    a = v.reshape(-1, 16).T.astype(np.int16)
    return np.tile(a, (8, 1))


def _bn_affine(g, b, m, v):
    s = g / np.sqrt(v + EPS)
    return s.astype(np.float32), (b - m * s).astype(np.float32)


def _prep_shared(inp):
    sh = {}
    w1 = np.asarray(inp["w1_w"], np.float32)   # [256, 304, 9]
    w1p = np.zeros((128, KK * 3 * C), np.float32)
    for j in range(KK):
        for kc in range(3):
            ci0 = kc * 128
            ncid = min(128, C1R - ci0)
            if ncid > 0:
                blk = w1[:, ci0:ci0 + ncid, j].T  # [ncid, 256]
                w1p[:ncid, (j * 3 + kc) * C:(j * 3 + kc) * C + C] = blk
    sh["w1p"] = w1p.astype(BF16)

    w2 = np.asarray(inp["w2_w"], np.float32)   # [256, 256, 9]
    w2p = np.zeros((128, KK * 2 * C), np.float32)
    for j in range(KK):
        for kc in range(2):
            blk = w2[:, kc * 128:(kc + 1) * 128, j].T
            w2p[:, (j * 2 + kc) * C:(j * 2 + kc) * C + C] = blk
    sh["w2p"] = w2p.astype(BF16)

    sh["c1"] = np.asarray(inp["conv1_w"], np.float32).T.astype(BF16)  # [128,48]

    ow = np.asarray(inp["out_w"], np.float32)  # [13, 256]
    owp = np.zeros((128, 2 * CLS), np.float32)
    for g in range(2):
        owp[:, g * CLS:(g + 1) * CLS] = ow[:, g * 128:(g + 1) * 128].T
    sh["outw"] = owp.astype(BF16)

    sh["eye"] = np.eye(128, dtype=BF16)

    bnv = np.zeros((128, 11), np.float32)
    s1, b1 = _bn_affine(inp["bn1_g"], inp["bn1_b"], inp["bn1_m"], inp["bn1_v"])
    bnv[:48, 0], bnv[:48, 1] = s1, b1
    sc, bc = _bn_affine(inp["bnc1_g"], inp["bnc1_b"], inp["bnc1_m"], inp["bnc1_v"])
    bc = bc + np.asarray(inp["w1_b"], np.float32) * sc
    for g in range(2):
        bnv[:, 2 + g] = sc[g * 128:(g + 1) * 128] / 3.0
        bnv[:, 4 + g] = bc[g * 128:(g + 1) * 128]
    sc2, bc2 = _bn_affine(inp["bnc2_g"], inp["bnc2_b"], inp["bnc2_m"], inp["bnc2_v"])
    bc2 = bc2 + np.asarray(inp["w2_b"], np.float32) * sc2
    for g in range(2):
        bnv[:, 6 + g] = sc2[g * 128:(g + 1) * 128] / 3.0
        bnv[:, 8 + g] = bc2[g * 128:(g + 1) * 128]
    bnv[:CLS, 10] = np.asarray(inp["out_b"], np.float32)
    sh["bnvec"] = bnv

    # interp tables: per chunk 1024 wrapped idxs (512 of i0, 512 of i1)
    pos = np.arange(N, dtype=np.float64) * ((LL - 1) / (N - 1))
    i0 = np.floor(pos).astype(np.int64)
    i1 = np.minimum(i0 + 1, LL - 1)
    t = (pos - i0).astype(np.float32)
    icols = []
    for ch in range(NTS0):
        s = slice(ch * NPT, (ch + 1) * NPT)
        icols.append(_wrap16(np.concatenate([i0[s], i1[s]])))
    sh["iidx"] = np.concatenate(icols, axis=1)           # [128, NTS0*64]
    tt = np.zeros((128, NTS0 * 4), np.float32)
    for ch in range(NTS0):
        for s in range(4):
            tt[:, ch * 4 + s] = t[ch * NPT + s * 128: ch * NPT + (s + 1) * 128]
    sh["tt"] = tt
    return sh


def _prep_core(inp, c):
    b, q = c // GRP, c % GRP
    pc = {}
    x = np.asarray(inp["x"], np.float32)
    pc["xt"] = np.ascontiguousarray(x[b].T).astype(BF16)          # [8192, 256]
    pc["llf"] = np.asarray(inp["low_level_feat"], np.float32)[b].astype(BF16)

    rot = np.asarray(inp["rotations"], np.int64)[:, b, :]          # [O, N]

    # geometry weights in permuted space, OOB taps zeroed
    coords = np.asarray(inp["coords"], np.float32)[b]              # [3, N]
    dist = np.asarray(inp["distances"], np.float32)[b]             # [O, N]
    wall = np.zeros((O, KK, N), np.float32)
    ar = np.arange(N)
    for o in range(O):
        co = coords[:, rot[o]]                                     # [3, N]
        d = dist[o]
        dp = np.pad(d, (PAD, PAD))
        cp = np.pad(co, ((0, 0), (PAD, PAD)))
        for j in range(KK):
            dd = (dp[j:j + N] - d) ** 2
            dc = ((cp[:, j:j + N] - co) ** 2).sum(0)
            w = np.exp(-(dd + dc))
            pin = ar + j - PAD
            w[(pin < 0) | (pin >= N)] = 0.0
            wall[o, j] = w
    # per-core chunk-blocked, pre-broadcast to 128 partitions
    wgt = np.zeros((O * NT, KK * NPT), np.float32)
    for o in range(O):
        for tch in range(NT):
            base = q * QN + tch * NPT
            wgt[o * NT + tch] = wall[o, :, base:base + NPT].reshape(KK * NPT)
    pc["wgt"] = np.ascontiguousarray(
        np.broadcast_to(wgt.astype(BF16)[:, None, :], (O * NT, 128, KK * NPT)))

    # gather window point-ids per (order, tile): used for L1 (from h_t) and
    # L2 (from zf1) gathers; plus scatter ids (the 512 owned slots' points)
    mar = np.arange(NGI)
    g1cols, sccols = [], []
    for o in range(O):
        for tch in range(NT):
            base = q * QN + tch * NPT
            pp = base - PAD + mar
            valid = (pp >= 0) & (pp < N)
            ppc = np.clip(pp, 0, N - 1)
            g1cols.append(_wrap16(np.where(valid, rot[o][ppc], 0)))
            sccols.append(_wrap16(rot[o][base:base + NPT]))
    pc["g1i"] = np.concatenate(g1cols, axis=1)    # [128, O*NT*40]
    pc["sci"] = np.concatenate(sccols, axis=1)    # [128, O*NT*32]

    # final gathers read the core's own ReduceScatter quarter r2 [QN, C]:
    # local row = global point - q*QN = tch*NPT + i
    fcols = []
    for tch in range(NTF):
        fcols.append(_wrap16(tch * NPT + np.arange(NPT)))
    pc["fci"] = np.concatenate(fcols, axis=1)     # [128, NTF*32]
    return pc


# ----------------------------------------------------------------------------
# device program
# ----------------------------------------------------------------------------

def _build_nc():
    import concourse.bacc as bacc
    import concourse.bass as bass
    import concourse.tile as tile
    import concourse.mybir as mybir
    from concourse.library_config import mlp

    dt = mybir.dt
    AF = mybir.ActivationFunctionType
    nocc = os.environ.get("KNOCC", "0") == "1"
    noag = nocc or os.environ.get("KNOAG", "0") == "1"
    nc = bacc.Bacc("TRN2", target_bir_lowering=False, debug=False,
                   num_devices=1 if nocc else NCORES, num_swdge_queues=4)

    def EIN(name, shape, dty):
        return nc.dram_tensor(name, list(shape), dty, kind="ExternalInput")

    xt = EIN("xt", [LL, C], dt.bfloat16)
    llf = EIN("llf", [128, N], dt.bfloat16)
    w1p = EIN("w1p", [128, KK * 3 * C], dt.bfloat16)
    w2p = EIN("w2p", [128, KK * 2 * C], dt.bfloat16)
    c1 = EIN("c1", [128, 48], dt.bfloat16)
    outw = EIN("outw", [128, 2 * CLS], dt.bfloat16)
    eye = EIN("eye", [128, 128], dt.bfloat16)
    bnvec = EIN("bnvec", [128, 11], dt.float32)
    wgt = EIN("wgt", [O * NT, 128, KK * NPT], dt.bfloat16)
    g1i = EIN("g1i", [128, O * NT * (NGI // 16)], dt.int16)
    sci = EIN("sci", [128, O * NT * (NPT // 16)], dt.int16)
    fci = EIN("fci", [128, NTF * (NPT // 16)], dt.int16)
    iidx = EIN("iidx", [128, NTS0 * 64], dt.int16)
    tt = EIN("tt", [128, NTS0 * 4], dt.float32)

    out = nc.dram_tensor("out", [CLS, QN], dt.float32, kind="ExternalOutput")

    RG = [[0, 1, 2, 3], [4, 5, 6, 7]]
    NW = NGI // 16   # 40 idx cols per gather window
    NWS = NPT // 16  # 32 idx cols per scatter / final gather

    with tile.TileContext(nc) as tc:
        with (
            tc.tile_pool(name="dram", bufs=1, space="DRAM") as dpool,
            tc.tile_pool(name="res", bufs=1) as res,
            tc.tile_pool(name="wk", bufs=4) as wk,
            tc.tile_pool(name="wc", bufs=3) as wc,
            tc.tile_pool(name="ps", bufs=4, space="PSUM") as psp,
            tc.tile_pool(name="pt", bufs=4, space="PSUM") as ptp,
        ):
            nc.gpsimd.load_library(mlp)

            h_t = dpool.tile([N, CH1], dt.bfloat16, tag="h")
            z1 = dpool.tile([N, C], dt.bfloat16, tag="z1")
            z2 = dpool.tile([N, C], dt.bfloat16, tag="z2")
            r2 = dpool.tile([QN, C], dt.bfloat16, tag="r2")
            zf1 = dpool.tile([N, C], dt.bfloat16, tag="zf1")

            # resident SBUF constants
            def LOAD(src, shape, dty, tag):
                tl = res.tile(shape, dty, tag=tag, name=tag)
                nc.sync.dma_start(tl[:], src[:])
                return tl
            w1s = LOAD(w1p, [128, KK * 3 * C], dt.bfloat16, "w1s")
            w2s = LOAD(w2p, [128, KK * 2 * C], dt.bfloat16, "w2s")
            c1s = LOAD(c1, [128, 48], dt.bfloat16, "c1s")
            ows = LOAD(outw, [128, 2 * CLS], dt.bfloat16, "ows")
            eys = LOAD(eye, [128, 128], dt.bfloat16, "eys")
            bns = LOAD(bnvec, [128, 11], dt.float32, "bns")
            g1s = LOAD(g1i, [128, O * NT * NW], dt.int16, "g1s")
            scs = LOAD(sci, [128, O * NT * NWS], dt.int16, "scs")
            fcs = LOAD(fci, [128, NTF * NWS], dt.int16, "fcs")
            iis = LOAD(iidx, [128, NTS0 * 64], dt.int16, "iis")
            tts = LOAD(tt, [128, NTS0 * 4], dt.float32, "tts")

            def rows_pm(dram_tile, base, nrows, rowlen):
                """point-major SBUF tile [128, nrows//128, rowlen] <-> dram rows."""
                return bass.AP(dram_tile.tensor, base * rowlen,
                               [[rowlen, 128], [128 * rowlen, nrows // 128],
                                [1, rowlen]])

            # ---------------- zero-init z1/z2 (overlaps stage 0) -----------
            ZR = 16  # 2048 rows per DMA
            zrs = res.tile([128, ZR, C], dt.bfloat16, tag="zrs", name="zrs")
            nc.vector.memset(zrs[:], 0)
            for zt_ in (z1, z2):
                for k in range(N // (128 * ZR)):
                    nc.sync.dma_start(rows_pm(zt_, k * 128 * ZR, 128 * ZR, C),
                                      zrs[:])

            # ---------------- stage 0: build h ----------------
            for ch in range(NTS0):
                xg = wk.tile([128, 8, C], dt.bfloat16, tag="xg")
                nc.gpsimd.dma_gather(
                    xg[:, :, :], xt[:, :], iis[:, ch * 64:(ch + 1) * 64],
                    8 * 128, 8 * 128, C, transpose=False, queue_num=ch % 4)
                hrow = wk.tile([128, 4, CH1], dt.bfloat16, tag="hrow")
                xd = wk.tile([128, 4, C], dt.bfloat16, tag="xd")
                nc.vector.tensor_sub(xd[:], xg[:, 4:8, :], xg[:, 0:4, :])
                for s in range(4):
                    nc.vector.tensor_scalar_mul(
                        xd[:, s, :], xd[:, s, :], tts[:, ch * 4 + s:ch * 4 + s + 1])
                nc.vector.tensor_add(hrow[:, :, 0:C], xg[:, 0:4, :], xd[:])

                lsb = wk.tile([128, NPT], dt.bfloat16, tag="lsb")
                nc.sync.dma_start(lsb[:], llf[:, ch * NPT:(ch + 1) * NPT])
                p48 = psp.tile([48, NPT], dt.float32, tag="pc")
                nc.tensor.matmul(p48[:], c1s[:], lsb[:], start=True, stop=True)
                low = wk.tile([48, NPT], dt.bfloat16, tag="low")
                nc.scalar.activation(low[:], p48[:], AF.Relu,
                                     bias=bns[:48, 1:2], scale=bns[:48, 0:1])
                for s in range(4):
                    ptt = ptp.tile([128, 48], dt.bfloat16, tag="pt")
                    nc.tensor.transpose(ptt[:], low[:48, s * 128:(s + 1) * 128],
                                        eys[:48, :48])
                    nc.scalar.activation(hrow[:, s, C:C + 48], ptt[:], AF.Copy)
                nc.vector.memset(hrow[:, :, C + 48:CH1], 0)
                nc.sync.dma_start(rows_pm(h_t, ch * NPT, NPT, CH1), hrow[:])

            # ---------------- conv layer helper ----------------
            def conv_layer(zdst, wsb_pack, nkc, gather_one, after=None):
                for o in range(O):
                    for tch in range(NT):
                        blk = o * NT + tch
                        hx = gather_one(o, tch)
                        wsb = wc.tile([128, KK * NPT], dt.bfloat16, tag="wsb")
                        nc.sync.dma_start(wsb[:], wgt[blk, :, :])
                        pg = [psp.tile([128, NPT], dt.float32, tag="pc",
                                       name=f"pg{g}") for g in range(2)]
                        for j in range(KK):
                            xw = wk.tile([128, nkc, NPT], dt.bfloat16, tag="xw")
                            for kc in range(nkc):
                                nc.vector.tensor_mul(
                                    xw[:, kc, :], hx[:, kc, j:j + NPT],
                                    wsb[:, j * NPT:(j + 1) * NPT])
                            for g in range(2):
                                for kc in range(nkc):
                                    wsl = wsb_pack[:, ((j * nkc + kc) * C + g * 128):
                                                   ((j * nkc + kc) * C + g * 128 + 128)]
                                    nc.tensor.matmul(
                                        pg[g][:], wsl, xw[:, kc, :],
                                        start=(j == 0 and kc == 0),
                                        stop=(j == KK - 1 and kc == nkc - 1))
                        ysb = wk.tile([128, 2, NPT], dt.bfloat16, tag="ysb")
                        for g in range(2):
                            nc.scalar.activation(ysb[:, g, :], pg[g][:], AF.Copy)
                        yT = wk.tile([128, 4, C], dt.bfloat16, tag="yT")
                        for g in range(2):
                            for s in range(4):
                                ptt = ptp.tile([128, 128], dt.bfloat16, tag="pt")
                                nc.tensor.transpose(
                                    ptt[:], ysb[:, g, s * 128:(s + 1) * 128], eys[:])
                                nc.scalar.activation(
                                    yT[:, s, g * 128:(g + 1) * 128], ptt[:], AF.Copy)
                        nc.gpsimd.dma_scatter_add(
                            zdst[:, :], yT[:, :, :],
                            scs[:, blk * NWS:(blk + 1) * NWS], NPT, NPT, C,
                            queue_num=blk % 4)
                if after is not None:
                    after()

            # L1: transpose-gather padded-384 rows of h -> channel-major
            def gather_l1(o, tch):
                blk = o * NT + tch
                hx = wk.tile([128, 3, NGI], dt.bfloat16, tag="g1hx")
                nc.gpsimd.dma_gather(
                    hx[:, :, :], h_t[:, :], g1s[:, blk * NW:(blk + 1) * NW],
                    NGI, NGI, CH1, transpose=True, queue_num=blk % 4)
                return hx

            def ar1():
                if noag:
                    return
                nc.gpsimd.collective_compute(
                    "AllReduce", mybir.AluOpType.add, replica_groups=RG,
                    ins=[z1.opt()], outs=[zf1.opt()])

            def rs2():
                if noag:
                    return
                nc.gpsimd.collective_compute(
                    "ReduceScatter", mybir.AluOpType.add, replica_groups=RG,
                    ins=[z2.opt()], outs=[r2.opt()])

            conv_layer(z1, w1s, 3, gather_l1, after=ar1)

            # L2: one transpose-gather from zf1 (same point-id table as L1),
            # then bn1+relu on the scalar engine
            def gather_l2(o, tch):
                blk = o * NT + tch
                g2t = wk.tile([128, 2, NGI], dt.bfloat16, tag="g2t")
                nc.gpsimd.dma_gather(
                    g2t[:, :, :], zf1[:, :], g1s[:, blk * NW:(blk + 1) * NW],
                    NGI, NGI, C, transpose=True, queue_num=blk % 4)
                hx = wk.tile([128, 2, NGI], dt.bfloat16, tag="g2hx")
                for g in range(2):
                    nc.scalar.activation(hx[:, g, :], g2t[:, g, :], AF.Relu,
                                         bias=bns[:, 4 + g:5 + g],
                                         scale=bns[:, 2 + g:3 + g])
                return hx

            conv_layer(z2, w2s, 2, gather_l2, after=rs2)

            # ---------------- final: bn2+relu+proj ----------------
            for tch in range(NTF):
                g3t = wk.tile([128, 2, NPT], dt.bfloat16, tag="g3t")
                nc.gpsimd.dma_gather(
                    g3t[:, :, :], r2[:, :], fcs[:, tch * NWS:(tch + 1) * NWS],
                    NPT, NPT, C, transpose=True, queue_num=tch % 4)
                h2 = wk.tile([128, 2, NPT], dt.bfloat16, tag="h2")
                for g in range(2):
                    nc.scalar.activation(h2[:, g, :], g3t[:, g, :], AF.Relu,
                                         bias=bns[:, 8 + g:9 + g],
                                         scale=bns[:, 6 + g:7 + g])
                pf = psp.tile([CLS, NPT], dt.float32, tag="pc")
                for g in range(2):
                    nc.tensor.matmul(pf[:], ows[:, g * CLS:(g + 1) * CLS],
                                     h2[:, g, :], start=(g == 0), stop=(g == 1))
                osb = wk.tile([CLS, NPT], dt.float32, tag="osb")
                nc.vector.tensor_scalar_add(osb[:], pf[:], bns[:CLS, 10:11])
                nc.sync.dma_start(out[:, tch * NPT:(tch + 1) * NPT], osb[:])

    nc.compile()
    return nc


# ----------------------------------------------------------------------------
# entry point
# ----------------------------------------------------------------------------

def kernel(**inputs):
    from concourse.bass_utils import run_bass_kernel_spmd

    if "nc" not in _CACHE:
        _CACHE["nc"] = _build_nc()
    nc = _CACHE["nc"]

    sh = _prep_shared(inputs)
    in_maps = []
    for c in range(NCORES):
        m = dict(sh)
        m.update(_prep_core(inputs, c))
        in_maps.append(m)

    res = run_bass_kernel_spmd(nc, in_maps, core_ids=list(range(NCORES)))
    outs = res.results
    full = np.zeros((B, CLS, N), np.float32)
    for c in range(NCORES):
        b, q = c // GRP, c % GRP
        full[b, :, q * QN:(q + 1) * QN] = outs[c]["out"]
    return full


# revision 20
# speedup vs baseline: 1.7988x; 1.2093x over previous
"""Trainium2 Bass kernel for nn_Decoder_17076789969159 (gnn_message_passing).

Sharding: data-parallel over batch (2 groups of 4 cores); within a group the
permuted point axis of each space-filling-curve order is split in 4 contiguous
chunks.

v3 dataflow (per group): conv outputs of all 3 orders are scatter-added
(SWDGE dma_scatter_add) into a per-point partial-sum tensor z [N, C]; a
ReduceScatter+AllGather pair over the group completes z = sum over orders of
y_o at each original point. The next layer does ONE transpose-mode dma_gather
per tile from z (channel-major landing, no PE transposes on the gather side)
and applies bn+relu post-gather on the Scalar engine. The final stage reads
z2 at contiguous point rows (HBM-friendly) with one gather per tile.

This minimizes random-row HBM traffic (the real bottleneck): one scatter +
one gather per point per layer instead of 3 slab gathers + 3 AllGathers.

Self-contained: hardcodes all shapes from the problem spec.
"""

import os
import numpy as np
import ml_dtypes

BF16 = ml_dtypes.bfloat16

# Problem shapes (hardcoded per contract)
B, N, LL, O, KK, PAD = 2, 32768, 8192, 3, 9, 4
C = 256          # conv output channels
C1R = 304        # conv1 input channels (256 xi + 48 low)
CH1 = 384        # padded h row (3 * 128)
CLS = 13
NCORES, GRP = 8, 4
QN = N // GRP            # 8192 permuted positions per core per order
NPT = 512                # points per conv tile
NT = QN // NPT           # 16 conv tiles per (order) per core
NTS0 = N // NPT          # 64 stage0 tiles (full batch, replicated in group)
NTF = QN // NPT          # 16 final tiles (core's original-index quarter)
NGI = 640                # gathered window, padded to 128 multiple
EPS = 1e-5

_CACHE = {}


# ----------------------------------------------------------------------------
# host-side preparation
# ----------------------------------------------------------------------------

def _wrap16(vals):
    """index vector of length n (mult of 16) -> [128, n//16] int16 wrapped.

    The 16-partition wrap is replicated down all 128 partitions: each of the
    8 GpSimd Q7 cores reads its own 16-partition stripe on hardware.
    """
    v = np.asarray(vals, np.int64)
    a = v.reshape(-1, 16).T.astype(np.int16)
    return np.tile(a, (8, 1))


def _bn_affine(g, b, m, v):
    s = g / np.sqrt(v + EPS)
    return s.astype(np.float32), (b - m * s).astype(np.float32)


def _prep_shared(inp):
    sh = {}
    w1 = np.asarray(inp["w1_w"], np.float32)   # [256, 304, 9]
    w1p = np.zeros((128, KK * 3 * C), np.float32)
    for j in range(KK):
        for kc in range(3):
            ci0 = kc * 128
            ncid = min(128, C1R - ci0)
            if ncid > 0:
                blk = w1[:, ci0:ci0 + ncid, j].T  # [ncid, 256]
                w1p[:ncid, (j * 3 + kc) * C:(j * 3 + kc) * C + C] = blk
    sh["w1p"] = w1p.astype(BF16)

    w2 = np.asarray(inp["w2_w"], np.float32)   # [256, 256, 9]
    w2p = np.zeros((128, KK * 2 * C), np.float32)
    for j in range(KK):
        for kc in range(2):
            blk = w2[:, kc * 128:(kc + 1) * 128, j].T
            w2p[:, (j * 2 + kc) * C:(j * 2 + kc) * C + C] = blk
    sh["w2p"] = w2p.astype(BF16)

    sh["c1"] = np.asarray(inp["conv1_w"], np.float32).T.astype(BF16)  # [128,48]

    ow = np.asarray(inp["out_w"], np.float32)  # [13, 256]
    owp = np.zeros((128, 2 * CLS), np.float32)
    for g in range(2):
        owp[:, g * CLS:(g + 1) * CLS] = ow[:, g * 128:(g + 1) * 128].T
    sh["outw"] = owp.astype(BF16)

    sh["eye"] = np.eye(128, dtype=BF16)

    bnv = np.zeros((128, 11), np.float32)
    s1, b1 = _bn_affine(inp["bn1_g"], inp["bn1_b"], inp["bn1_m"], inp["bn1_v"])
    bnv[:48, 0], bnv[:48, 1] = s1, b1
    sc, bc = _bn_affine(inp["bnc1_g"], inp["bnc1_b"], inp["bnc1_m"], inp["bnc1_v"])
    bc = bc + np.asarray(inp["w1_b"], np.float32) * sc
    for g in range(2):
        bnv[:, 2 + g] = sc[g * 128:(g + 1) * 128] / 3.0
        bnv[:, 4 + g] = bc[g * 128:(g + 1) * 128]
    sc2, bc2 = _bn_affine(inp["bnc2_g"], inp["bnc2_b"], inp["bnc2_m"], inp["bnc2_v"])
    bc2 = bc2 + np.asarray(inp["w2_b"], np.float32) * sc2
    for g in range(2):
        bnv[:, 6 + g] = sc2[g * 128:(g + 1) * 128] / 3.0
        bnv[:, 8 + g] = bc2[g * 128:(g + 1) * 128]
    bnv[:CLS, 10] = np.asarray(inp["out_b"], np.float32)
    sh["bnvec"] = bnv

    # interp tables: per chunk 1024 wrapped idxs (512 of i0, 512 of i1)
    pos = np.arange(N, dtype=np.float64) * ((LL - 1) / (N - 1))
    i0 = np.floor(pos).astype(np.int64)
    i1 = np.minimum(i0 + 1, LL - 1)
    t = (pos - i0).astype(np.float32)
    icols = []
    for ch in range(NTS0):
        s = slice(ch * NPT, (ch + 1) * NPT)
        icols.append(_wrap16(np.concatenate([i0[s], i1[s]])))
    sh["iidx"] = np.concatenate(icols, axis=1)           # [128, NTS0*64]
    tt = np.zeros((128, NTS0 * 4), np.float32)
    for ch in range(NTS0):
        for s in range(4):
            tt[:, ch * 4 + s] = t[ch * NPT + s * 128: ch * NPT + (s + 1) * 128]
    sh["tt"] = tt
    return sh


def _prep_core(inp, c):
    b, q = c // GRP, c % GRP
    pc = {}
    x = np.asarray(inp["x"], np.float32)
    pc["xt"] = np.ascontiguousarray(x[b].T).astype(BF16)          # [8192, 256]
    pc["llf"] = np.asarray(inp["low_level_feat"], np.float32)[b].astype(BF16)

    rot = np.asarray(inp["rotations"], np.int64)[:, b, :]          # [O, N]

    # geometry weights in permuted space, OOB taps zeroed
    coords = np.asarray(inp["coords"], np.float32)[b]              # [3, N]
    dist = np.asarray(inp["distances"], np.float32)[b]             # [O, N]
    wall = np.zeros((O, KK, N), np.float32)
    ar = np.arange(N)
    for o in range(O):
        co = coords[:, rot[o]]                                     # [3, N]
        d = dist[o]
        dp = np.pad(d, (PAD, PAD))
        cp = np.pad(co, ((0, 0), (PAD, PAD)))
        for j in range(KK):
            dd = (dp[j:j + N] - d) ** 2
            dc = ((cp[:, j:j + N] - co) ** 2).sum(0)
            w = np.exp(-(dd + dc))
            pin = ar + j - PAD
            w[(pin < 0) | (pin >= N)] = 0.0
            wall[o, j] = w
    # per-core chunk-blocked, pre-broadcast to 128 partitions
    wgt = np.zeros((O * NT, KK * NPT), np.float32)
    for o in range(O):
        for tch in range(NT):
            base = q * QN + tch * NPT
            wgt[o * NT + tch] = wall[o, :, base:base + NPT].reshape(KK * NPT)
    pc["wgt"] = np.ascontiguousarray(
        np.broadcast_to(wgt.astype(BF16)[:, None, :], (O * NT, 128, KK * NPT)))

    # gather window point-ids per (order, tile): used for L1 (from h_t) and
    # L2 (from zf1) gathers; plus scatter ids (the 512 owned slots' points)
    mar = np.arange(NGI)
    g1cols, sccols = [], []
    for o in range(O):
        for tch in range(NT):
            base = q * QN + tch * NPT
            pp = base - PAD + mar
            valid = (pp >= 0) & (pp < N)
            ppc = np.clip(pp, 0, N - 1)
            g1cols.append(_wrap16(np.where(valid, rot[o][ppc], 0)))
            sccols.append(_wrap16(rot[o][base:base + NPT]))
    pc["g1i"] = np.concatenate(g1cols, axis=1)    # [128, O*NT*40]
    pc["sci"] = np.concatenate(sccols, axis=1)    # [128, O*NT*32]

    # final gathers read the core's own ReduceScatter quarter r2 [QN, C]:
    # local row = global point - q*QN = tch*NPT + i
    fcols = []
    for tch in range(NTF):
        fcols.append(_wrap16(tch * NPT + np.arange(NPT)))
    pc["fci"] = np.concatenate(fcols, axis=1)     # [128, NTF*32]
    return pc


# ----------------------------------------------------------------------------
# device program
# ----------------------------------------------------------------------------

def _build_nc():
    import concourse.bacc as bacc
    import concourse.bass as bass
    import concourse.tile as tile
    import concourse.mybir as mybir
    from concourse.library_config import mlp

    dt = mybir.dt
    AF = mybir.ActivationFunctionType
    nocc = os.environ.get("KNOCC", "0") == "1"
    noag = nocc or os.environ.get("KNOAG", "0") == "1"
    nc = bacc.Bacc("TRN2", target_bir_lowering=False, debug=False,
                   num_devices=1 if nocc else NCORES, num_swdge_queues=4)

    def EIN(name, shape, dty):
        return nc.dram_tensor(name, list(shape), dty, kind="ExternalInput")

    xt = EIN("xt", [LL, C], dt.bfloat16)
    llf = EIN("llf", [128, N], dt.bfloat16)
    w1p = EIN("w1p", [128, KK * 3 * C], dt.bfloat16)
    w2p = EIN("w2p", [128, KK * 2 * C], dt.bfloat16)
    c1 = EIN("c1", [128, 48], dt.bfloat16)
    outw = EIN("outw", [128, 2 * CLS], dt.bfloat16)
    eye = EIN("eye", [128, 128], dt.bfloat16)
    bnvec = EIN("bnvec", [128, 11], dt.float32)
    wgt = EIN("wgt", [O * NT, 128, KK * NPT], dt.bfloat16)
    g1i = EIN("g1i", [128, O * NT * (NGI // 16)], dt.int16)
    sci = EIN("sci", [128, O * NT * (NPT // 16)], dt.int16)
    fci = EIN("fci", [128, NTF * (NPT // 16)], dt.int16)
    iidx = EIN("iidx", [128, NTS0 * 64], dt.int16)
    tt = EIN("tt", [128, NTS0 * 4], dt.float32)

    out = nc.dram_tensor("out", [CLS, QN], dt.float32, kind="ExternalOutput")

    RG = [[0, 1, 2, 3], [4, 5, 6, 7]]
    NW = NGI // 16   # 40 idx cols per gather window
    NWS = NPT // 16  # 32 idx cols per scatter / final gather

    with tile.TileContext(nc) as tc:
        with (
            tc.tile_pool(name="dram", bufs=1, space="DRAM") as dpool,
            tc.tile_pool(name="res", bufs=1) as res,
            tc.tile_pool(name="wk", bufs=4) as wk,
            tc.tile_pool(name="wc", bufs=3) as wc,
            tc.tile_pool(name="ps", bufs=4, space="PSUM") as psp,
            tc.tile_pool(name="pt", bufs=4, space="PSUM") as ptp,
        ):
            nc.gpsimd.load_library(mlp)

            h_t = dpool.tile([N, CH1], dt.bfloat16, tag="h")
            z1 = dpool.tile([N, C], dt.bfloat16, tag="z1")
            z2 = dpool.tile([N, C], dt.bfloat16, tag="z2")
            r2 = dpool.tile([QN, C], dt.bfloat16, tag="r2")
            zf1 = dpool.tile([N, C], dt.bfloat16, tag="zf1")

            # resident SBUF constants
            def LOAD(src, shape, dty, tag):
                tl = res.tile(shape, dty, tag=tag, name=tag)
                nc.sync.dma_start(tl[:], src[:])
                return tl
            w1s = LOAD(w1p, [128, KK * 3 * C], dt.bfloat16, "w1s")
            w2s = LOAD(w2p, [128, KK * 2 * C], dt.bfloat16, "w2s")
            c1s = LOAD(c1, [128, 48], dt.bfloat16, "c1s")
            ows = LOAD(outw, [128, 2 * CLS], dt.bfloat16, "ows")
            eys = LOAD(eye, [128, 128], dt.bfloat16, "eys")
            bns = LOAD(bnvec, [128, 11], dt.float32, "bns")
            g1s = LOAD(g1i, [128, O * NT * NW], dt.int16, "g1s")
            scs = LOAD(sci, [128, O * NT * NWS], dt.int16, "scs")
            fcs = LOAD(fci, [128, NTF * NWS], dt.int16, "fcs")
            iis = LOAD(iidx, [128, NTS0 * 64], dt.int16, "iis")
            tts = LOAD(tt, [128, NTS0 * 4], dt.float32, "tts")

            def rows_pm(dram_tile, base, nrows, rowlen):
                """point-major SBUF tile [128, nrows//128, rowlen] <-> dram rows."""
                return bass.AP(dram_tile.tensor, base * rowlen,
                               [[rowlen, 128], [128 * rowlen, nrows // 128],
                                [1, rowlen]])

            # ---------------- zero-init z1/z2 (overlaps stage 0) -----------
            ZR = 16  # 2048 rows per DMA
            zrs = res.tile([128, ZR, C], dt.bfloat16, tag="zrs", name="zrs")
            nc.vector.memset(zrs[:], 0)
            for zt_ in (z1, z2):
                for k in range(N // (128 * ZR)):
                    nc.sync.dma_start(rows_pm(zt_, k * 128 * ZR, 128 * ZR, C),
                                      zrs[:])

            # ---------------- stage 0: build h ----------------
            for ch in range(NTS0):
                xg = wk.tile([128, 8, C], dt.bfloat16, tag="xg")
                nc.gpsimd.dma_gather(
                    xg[:, :, :], xt[:, :], iis[:, ch * 64:(ch + 1) * 64],
                    8 * 128, 8 * 128, C, transpose=False, queue_num=ch % 4)
                hrow = wk.tile([128, 4, CH1], dt.bfloat16, tag="hrow")
                xd = wk.tile([128, 4, C], dt.bfloat16, tag="xd")
                nc.vector.tensor_sub(xd[:], xg[:, 4:8, :], xg[:, 0:4, :])
                for s in range(4):
                    nc.vector.tensor_scalar_mul(
                        xd[:, s, :], xd[:, s, :], tts[:, ch * 4 + s:ch * 4 + s + 1])
                nc.vector.tensor_add(hrow[:, :, 0:C], xg[:, 0:4, :], xd[:])

                lsb = wk.tile([128, NPT], dt.bfloat16, tag="lsb")
                nc.sync.dma_start(lsb[:], llf[:, ch * NPT:(ch + 1) * NPT])
                p48 = psp.tile([48, NPT], dt.float32, tag="pc")
                nc.tensor.matmul(p48[:], c1s[:], lsb[:], start=True, stop=True)
                low = wk.tile([48, NPT], dt.bfloat16, tag="low")
                nc.scalar.activation(low[:], p48[:], AF.Relu,
                                     bias=bns[:48, 1:2], scale=bns[:48, 0:1])
                for s in range(4):
                    ptt = ptp.tile([128, 48], dt.bfloat16, tag="pt")
                    nc.tensor.transpose(ptt[:], low[:48, s * 128:(s + 1) * 128],
                                        eys[:48, :48])
                    nc.scalar.activation(hrow[:, s, C:C + 48], ptt[:], AF.Copy)
                nc.vector.memset(hrow[:, :, C + 48:CH1], 0)
                nc.sync.dma_start(rows_pm(h_t, ch * NPT, NPT, CH1), hrow[:])

            # ---------------- conv layer helper ----------------
            def conv_layer(zdst, wsb_pack, nkc, gather_one, after=None):
                for o in range(O):
                    for tch in range(NT):
                        blk = o * NT + tch
                        hx = gather_one(o, tch)
                        wsb = wc.tile([128, KK * NPT], dt.bfloat16, tag="wsb")
                        nc.sync.dma_start(wsb[:], wgt[blk, :, :])
                        pg = [psp.tile([128, NPT], dt.float32, tag="pc",
                                       name=f"pg{g}") for g in range(2)]
                        for j in range(KK):
                            xw = wk.tile([128, nkc, NPT], dt.bfloat16, tag="xw")
                            for kc in range(nkc):
                                nc.vector.tensor_mul(
                                    xw[:, kc, :], hx[:, kc, j:j + NPT],
                                    wsb[:, j * NPT:(j + 1) * NPT])
                            for g in range(2):
                                for kc in range(nkc):
                                    wsl = wsb_pack[:, ((j * nkc + kc) * C + g * 128):
                                                   ((j * nkc + kc) * C + g * 128 + 128)]
                                    nc.tensor.matmul(
                                        pg[g][:], wsl, xw[:, kc, :],
                                        start=(j == 0 and kc == 0),
                                        stop=(j == KK - 1 and kc == nkc - 1))
                        ysb = wk.tile([128, 2, NPT], dt.bfloat16, tag="ysb")
                        for g in range(2):
                            nc.scalar.activation(ysb[:, g, :], pg[g][:], AF.Copy)
                        yT = wk.tile([128, 4, C], dt.bfloat16, tag="yT")
                        for g in range(2):
                            for s in range(4):
                                ptt = ptp.tile([128, 128], dt.bfloat16, tag="pt")
                                nc.tensor.transpose(
                                    ptt[:], ysb[:, g, s * 128:(s + 1) * 128], eys[:])
                                nc.scalar.activation(
                                    yT[:, s, g * 128:(g + 1) * 128], ptt[:], AF.Copy)
                        nc.gpsimd.dma_scatter_add(
                            zdst[:, :], yT[:, :, :],
                            scs[:, blk * NWS:(blk + 1) * NWS], NPT, NPT, C,
                            queue_num=blk % 4)
                if after is not None:
                    after()

            # L1: transpose-gather padded-384 rows of h -> channel-major
            def gather_l1(o, tch):
                blk = o * NT + tch
                hx = wk.tile([128, 3, NGI], dt.bfloat16, tag="g1hx")
                nc.gpsimd.dma_gather(
                    hx[:, :, :], h_t[:, :], g1s[:, blk * NW:(blk + 1) * NW],
                    NGI, NGI, CH1, transpose=True, queue_num=blk % 4)
                return hx

            def ar1():
                if noag:
                    return
                nc.gpsimd.collective_compute(
                    "AllReduce", mybir.AluOpType.add, replica_groups=RG,
                    ins=[z1.opt()], outs=[zf1.opt()])

            def rs2():
                if noag:
                    return
                nc.gpsimd.collective_compute(
                    "ReduceScatter", mybir.AluOpType.add, replica_groups=RG,
                    ins=[z2.opt()], outs=[r2.opt()])

            conv_layer(z1, w1s, 3, gather_l1, after=ar1)

            # L2: one transpose-gather from zf1 (same point-id table as L1),
            # then bn1+relu on the scalar engine
            def gather_l2(o, tch):
                blk = o * NT + tch
                g2t = wk.tile([128, 2, NGI], dt.bfloat16, tag="g2t")
                nc.gpsimd.dma_gather(
                    g2t[:, :, :], zf1[:, :], g1s[:, blk * NW:(blk + 1) * NW],
                    NGI, NGI, C, transpose=True, queue_num=blk % 4)
                hx = wk.tile([128, 2, NGI], dt.bfloat16, tag="g2hx")
                for g in range(2):
                    nc.scalar.activation(hx[:, g, :], g2t[:, g, :], AF.Relu,
                                         bias=bns[:, 4 + g:5 + g],
                                         scale=bns[:, 2 + g:3 + g])
                return hx

            conv_layer(z2, w2s, 2, gather_l2, after=rs2)

            # ---------------- final: bn2+relu+proj ----------------
            for tch in range(NTF):
                g3t = wk.tile([128, 2, NPT], dt.bfloat16, tag="g3t")
                nc.gpsimd.dma_gather(
                    g3t[:, :, :], r2[:, :], fcs[:, tch * NWS:(tch + 1) * NWS],
                    NPT, NPT, C, transpose=True, queue_num=tch % 4)
                h2 = wk.tile([128, 2, NPT], dt.bfloat16, tag="h2")
                for g in range(2):
                    nc.scalar.activation(h2[:, g, :], g3t[:, g, :], AF.Relu,
                                         bias=bns[:, 8 + g:9 + g],
                                         scale=bns[:, 6 + g:7 + g])
                pf = psp.tile([CLS, NPT], dt.float32, tag="pc")
                for g in range(2):
                    nc.tensor.matmul(pf[:], ows[:, g * CLS:(g + 1) * CLS],
                                     h2[:, g, :], start=(g == 0), stop=(g == 1))
                osb = wk.tile([CLS, NPT], dt.float32, tag="osb")
                nc.vector.tensor_scalar_add(osb[:], pf[:], bns[:CLS, 10:11])
                nc.sync.dma_start(out[:, tch * NPT:(tch + 1) * NPT], osb[:])

    nc.compile()
    return nc


# ----------------------------------------------------------------------------
# entry point
# ----------------------------------------------------------------------------

def kernel(**inputs):
    from concourse.bass_utils import run_bass_kernel_spmd

    if "nc" not in _CACHE:
        _CACHE["nc"] = _build_nc()
    nc = _CACHE["nc"]

    sh = _prep_shared(inputs)
    in_maps = []
    for c in range(NCORES):
        m = dict(sh)
        m.update(_prep_core(inputs, c))
        in_maps.append(m)

    res = run_bass_kernel_spmd(nc, in_maps, core_ids=list(range(NCORES)))
    outs = res.results
    full = np.zeros((B, CLS, N), np.float32)
    for c in range(NCORES):
        b, q = c // GRP, c % GRP
        full[b, :, q * QN:(q + 1) * QN] = outs[c]["out"]
    return full
